# revision 46
# baseline (speedup 1.0000x reference)
"""BiMambaLayer Trainium2 kernel.

Sharding: 8 cores = batch(2) x direction(2) x head-half(2). Each core runs the
full L=2048 sequence of one (batch, direction) through 16 of the 32 heads of
that direction's Mamba2 block, plus the fused output projection restricted to
its 1024 d_inner channels. The gated-RMSNorm row scale commutes with the output
projections, so each core returns an unnormalized partial projection U and a
partial sum-of-squares s; the host combines:
    out[b] = x[b] + scale * sum_dir flip_d( r_d[:,None] * (U0 + U1) / 8 ),
    r_d = rsqrt((s0 + s1)/32 + eps).

Three phases over the whole sequence (so the activation table never thrashes
between the exp/ln set and the silu set, and the PE stays warm through dense
GEMM bursts):
  1) in_proj (fp8 DoubleRow) + dt pipeline for all 4 blocks of 512,
  2) z-proj (fp8 DoubleRow) + causal conv (fp8 diag matmuls) + all silus,
  3) 16 scan chunks of 128: head-shared C@B^T, per-head decay
     exp(l_i - l_j + ln dt_j) with the tri mask applied as min(exp,100)*g0m
     (masked entries overflow exp to +inf; min() tames them before the 0
     mask), Y^T = X^T.T@G^T + D_h*X^T + H^T.T@C''^T in PSUM, chunk-state
     recurrence on DVE, gating y8 = y*silu(z)/8 in fp8, U = y8 @ Mfused
     (fp8 DoubleRow).
fp8 scaling: in_proj weights x16 (undone at PSUM read), Mfused x64, y x1/8;
the host combine divides U by 8 and rescales s accordingly.
"""
import numpy as np

L = 2048
DM = 1024  # d_model
Q = 128  # scan chunk
NCH = L // Q  # 16 chunks
BLK = 512
NBLK = L // BLK  # 4
CPB = BLK // Q  # 4 chunks per block
NH = 16  # local heads
P = 64  # head dim
NST = 16  # state dim
EPS = 1e-5

_cache = {}


def _build_nc():
    import concourse.bass as bass
    import concourse.tile as tile
    import concourse.mybir as mybir
    from concourse import bacc
    from concourse.masks import make_identity
    from concourse.alu_op_type import AluOpType as alu

    f32 = mybir.dt.float32
    f16 = mybir.dt.float16
    bf16 = mybir.dt.bfloat16
    f8 = mybir.dt.float8e4
    AF = mybir.ActivationFunctionType
    DR = mybir.MatmulPerfMode.DoubleRow

    nc = bacc.Bacc(trn_type="TRN2")

    # ---- DRAM I/O (per-core shapes; host prepares layouts) ----
    xt = nc.dram_tensor("xt", [128, 8, L], f8, kind="ExternalInput")
    wt = nc.dram_tensor("wt", [128, 8, 2096], f8, kind="ExternalInput")
    mft = nc.dram_tensor("mft", [128, 8, DM], f8, kind="ExternalInput")
    cw = nc.dram_tensor("cw", [128, 36], f32, kind="ExternalInput")
    cb = nc.dram_tensor("cb", [128, 18], f32, kind="ExternalInput")
    hp = nc.dram_tensor("hp", [128, 18], f32, kind="ExternalInput")
    u = nc.dram_tensor("u", [L, DM], bf16, kind="ExternalOutput")
    s = nc.dram_tensor("s", [1, L], f32, kind="ExternalOutput")

    from contextlib import ExitStack

    with tile.TileContext(nc) as tc, ExitStack() as ctx:
        ep = ctx.enter_context
        const = ep(tc.tile_pool(name="const", bufs=1))
        seqp = ep(tc.tile_pool(name="seqp", bufs=1))
        statep = ep(tc.tile_pool(name="state", bufs=1))
        xtp = ep(tc.tile_pool(name="xtp", bufs=2))
        dtlp = ep(tc.tile_pool(name="dtlp", bufs=2))
        wcrp = ep(tc.tile_pool(name="wcrp", bufs=1))
        dscrp = ep(tc.tile_pool(name="dscrp", bufs=1, space="DRAM"))
        gp2 = ep(tc.tile_pool(name="gp2", bufs=2))
        chkp = ep(tc.tile_pool(name="chkp", bufs=3))
        gdecp = ep(tc.tile_pool(name="gdecp", bufs=3))
        gsbp = ep(tc.tile_pool(name="gsbp", bufs=3))
        y2p = ep(tc.tile_pool(name="y2p", bufs=2))
        y8p = ep(tc.tile_pool(name="y8p", bufs=2))
        y2blk = ep(tc.tile_pool(name="y2blk", bufs=1))
        pp_mm = ep(tc.tile_pool(name="pp_mm", bufs=1, space="PSUM"))
        pp_ytp = ep(tc.tile_pool(name="pp_ytp", bufs=3, space="PSUM"))
        pp_s = ep(tc.tile_pool(name="pp_s", bufs=2, space="PSUM"))
        pp_gd = ep(tc.tile_pool(name="pp_gd", bufs=2, space="PSUM"))
        if True:
            # ---------- constants / persistent ----------
            wt_sb = const.tile([128, 8, 2096], f8)
            nc.sync.dma_start(wt_sb, wt[:, :, :])
            mft_sb = const.tile([128, 8, DM], f8)
            nc.sync.dma_start(mft_sb, mft[:, :, :])
            cw_sb = const.tile([128, 36], f32)
            nc.sync.dma_start(cw_sb, cw[:, :])
            cb_sb = const.tile([128, 18], f32)
            nc.sync.dma_start(cb_sb, cb[:, :])
            hp_sb = const.tile([128, 18], f32)
            nc.sync.dma_start(hp_sb, hp[:, :])

            ident_b = const.tile([128, 128], bf16)
            make_identity(nc, ident_b)
            ident_f = const.tile([128, 128], f32)
            make_identity(nc, ident_f)
            # multiplicative mask, [j, i] coords: 1 where i >= j, 0 where i < j
            tril01 = const.tile([128, 128], bf16)
            nc.gpsimd.memset(tril01, 1.0)
            nc.gpsimd.affine_select(
                out=tril01, in_=tril01, compare_op=mybir.AluOpType.is_ge,
                fill=0.0, base=0, pattern=[[1, 128]], channel_multiplier=-1,
            )
            onesq = const.tile([128, 128], f32)
            nc.vector.memset(onesq, 1.0)
            onescol = const.tile([128, 1], f8)
            nc.vector.memset(onescol, 1.0)
            # conv diagonal weight tiles (fp8), built from cw columns
            convd = const.tile([128, 36, 128], f8)
            for j in range(36):
                nc.scalar.mul(convd[:, j, :], ident_b, cw_sb[:, j : j + 1])
            # persistent scan state: 4 head-groups (heads 4t+k at partitions
            # 32k..32k+16), ping-pong A/B
            stA = statep.tile([128, 4, P], f8, tag="stA")
            stB = statep.tile([128, 4, P], f8, tag="stB")
            nc.vector.memset(stA, 0.0)
            nc.vector.memset(stB, 0.0)
            st = [stA, stB]
            # chunk-decay per-partition scalars, [state-tile, chunk]
            texp_st = statep.tile([128, 4, NCH], f32, tag="texp")
            nc.vector.memset(texp_st, 0.0)

            # whole-sequence activations
            xsr = seqp.tile([128, 8, L + 3], f8)  # in_proj xs (+3 zero halo)
            bcr = seqp.tile([32, L + 3], f8)
            xs_sb = seqp.tile([128, 8, L], bf16)  # conv+silu out
            bct = seqp.tile([32, L], bf16)
            ct4 = seqp.tile([128, L], bf16)  # C rows replicated to 4 bases
            zs = seqp.tile([128, 8, L], f8)  # silu(z)
            c2t = seqp.tile([128, 4, L], f8)  # C * exp(l_h) per head
            nc.vector.memset(xsr[:, :, 0:3], 0.0)
            nc.vector.memset(bcr[:, 0:3], 0.0)
            # gdiff matmul operands (fp16, chunk-recentered l so fp16 is
            # accurate): partition 0/1 = {per-chunk data, constant}
            lsg = statep.tile([2, NH, Q], f16, tag="lsg")  # p0: l-lndt-T, p1: 1
            rsg = statep.tile([2, NH, Q], f16, tag="rsg")  # p0: -1, p1: l-T
            nc.vector.memset(lsg, 1.0)
            nc.vector.memset(rsg, -1.0)

            dscr = dscrp.tile([48, L], f32, tag="dscr")
            dscr2 = dscrp.tile([16, L], bf16, tag="dscr2")
            dscr3 = dscrp.tile([32, L], f16, tag="dscr3")

            # ================= phase 1: in_proj + dt =================
            for b in range(NBLK):
                bsl = slice(b * BLK, (b + 1) * BLK)
                bsl3 = slice(3 + b * BLK, 3 + (b + 1) * BLK)
                xtb = xtp.tile([128, 8, BLK], f8, tag="xtb")
                nc.sync.dma_start(xtb, xt[:, :, bsl])
                dt_sp = dtlp.tile([128, BLK], f32, tag="dtsp")
                for et in range(9):
                    m = 128 if et < 8 else 48
                    ecol = et * 128 if et < 8 else 1024
                    ps = [pp_mm, pp_ytp][et % 2].tile(
                        [128, BLK], f32, tag=["mm", "ytp"][et % 2])
                    for kj in range(4):
                        nc.tensor.matmul(
                            ps[:m, :], wt_sb[:, 2 * kj : 2 * kj + 2, ecol : ecol + m],
                            xtb[:, 2 * kj : 2 * kj + 2, :],
                            start=(kj == 0), stop=(kj == 3), perf_mode=DR,
                        )
                    if et < 8:
                        nc.scalar.mul(xsr[:, et, bsl3], ps, 0.0625)
                    else:
                        nc.scalar.mul(bcr[:, bsl3], ps[0:32, :], 0.0625)
                        nc.scalar.activation(
                            dt_sp[32:48, :], ps[32:48, :], AF.Exp,
                            bias=hp_sb[32:48, 0:1], scale=0.0625,
                        )
                        nc.vector.tensor_scalar_add(
                            dt_sp[32:48, :], dt_sp[32:48, :], 1.0
                        )
                        nc.scalar.activation(dt_sp[32:48, :], dt_sp[32:48, :], AF.Ln)
                # ---------- dt pipeline ----------
                lndt = dtlp.tile([128, BLK], f32, tag="lndt")
                lcm = dtlp.tile([128, BLK], f32, tag="lcm")
                wc2 = dtlp.tile([128, BLK], bf16, tag="wc2")
                nc.scalar.activation(lndt[32:48, :], dt_sp[32:48, :], AF.Ln)
                dtA = dtlp.tile([128, BLK], f32, tag="dtA")
                nc.vector.tensor_scalar_mul(
                    dtA[32:48, :], dt_sp[32:48, :], hp_sb[32:48, 1:2]
                )
                for cc in range(CPB):
                    qs = slice(cc * Q, (cc + 1) * Q)
                    nc.vector.tensor_tensor_scan(
                        lcm[32:48, qs], onesq[32:48, :], dtA[32:48, qs],
                        0.0, alu.mult, alu.add,
                    )
                nc.scalar.activation(wc2[32:48, :], lcm[32:48, :], AF.Exp)
                texp_cm = dtlp.tile([128, CPB, 1], f32, tag="texpcm")
                lv = lcm[32:48, :].rearrange("p (c q) -> p c q", q=Q)
                nc.scalar.activation(texp_cm[32:48, :, :], lv[:, :, 127:128], AF.Exp)
                # chunk-recentered l rows in fp16 for the gdiff matmuls
                lcmr = dtlp.tile([128, BLK], f16, tag="lcmr")
                lsubr = dtlp.tile([128, BLK], f16, tag="lsubr")
                nc.vector.tensor_tensor(
                    lcmr[32:48, :].rearrange("p (c q) -> p c q", q=Q),
                    lv, lv[:, :, 127:128].to_broadcast([16, CPB, Q]),
                    alu.subtract,
                )
                nc.vector.tensor_tensor(
                    lsubr[32:48, :], lcmr[32:48, :], lndt[32:48, :], alu.subtract
                )
                # bounce small per-block vectors through DRAM so they can be
                # partition-broadcast on the way back in
                nc.sync.dma_start(dscr3[0:16, bsl], lcmr[32:48, :])
                nc.sync.dma_start(dscr3[16:32, bsl], lsubr[32:48, :])
                nc.sync.dma_start(dscr2[:, bsl], wc2[32:48, :])
                nc.sync.dma_start(
                    dscr[32:48, b * CPB : (b + 1) * CPB],
                    texp_cm[32:48, :, :].rearrange("p c one -> p (c one)"),
                )
                for k in range(4):
                    nc.sync.dma_start(
                        texp_st[32 * k : 32 * k + 16, :, b * CPB : (b + 1) * CPB],
                        bass.AP(dscr.tensor,
                                dscr.offset + (32 + k) * L + b * CPB,
                                [[0, 16], [4 * L, 4], [1, CPB]]),
                    )

            # ================= phase 2: z + conv (all silus) =================
            tc.tile_set_cur_wait(1.0)
            for b in range(NBLK):
                bsl = slice(b * BLK, (b + 1) * BLK)
                xtb = xtp.tile([128, 8, BLK], f8, tag="xtb")
                nc.sync.dma_start(xtb, xt[:, :, bsl])
                for zt in range(8):
                    ps = [pp_mm, pp_ytp][zt % 2].tile(
                        [128, BLK], f32, tag=["mm", "ytp"][zt % 2])
                    for kj in range(4):
                        nc.tensor.matmul(
                            ps, wt_sb[:, 2 * kj : 2 * kj + 2, 1072 + zt * 128 : 1200 + zt * 128],
                            xtb[:, 2 * kj : 2 * kj + 2, :],
                            start=(kj == 0), stop=(kj == 3), perf_mode=DR,
                        )
                    nc.scalar.activation(zs[:, zt, bsl], ps, AF.Silu, scale=0.0625)
                for ct in range(9):
                    m = 128 if ct < 8 else 32
                    ps = [pp_mm, pp_ytp][ct % 2].tile(
                        [128, BLK], f32, tag=["mm", "ytp"][ct % 2])
                    for k in range(4):
                        a = b * BLK + k
                        mov = (xsr[:, ct, a : a + BLK] if ct < 8
                               else bcr[:, a : a + BLK])
                        nc.tensor.matmul(
                            ps[:m, :], convd[:m, ct * 4 + k, :m], mov,
                            start=(k == 0), stop=(k == 3),
                        )
                    dst = xs_sb[:, ct, bsl] if ct < 8 else bct[:, bsl]
                    nc.scalar.activation(
                        dst, ps[:m, :], AF.Silu, bias=cb_sb[:m, ct : ct + 1]
                    )
                # C rows replicated to the four 32-aligned bases
                for k4 in range(4):
                    nc.sync.dma_start(ct4[32 * k4 : 32 * k4 + 16, bsl], bct[16:32, bsl])
                # C'' = C * exp(l_h) per head
                wc2rep = wcrp.tile([128, 4, BLK], bf16, tag="wc2rep")
                _qw = [nc.scalar, nc.gpsimd, nc.sync, nc.gpsimd]
                for k in range(4):
                    _qw[k].dma_start(
                        wc2rep[32 * k : 32 * k + 16, :, :],
                        bass.AP(dscr2.tensor, dscr2.offset + k * L + b * BLK,
                                [[0, 16], [4 * L, 4], [1, BLK]]),
                    )
                nc.vector.tensor_tensor(
                    c2t[:, :, bsl],
                    ct4[:, bsl].rearrange("p (one c) -> p one c", one=1)
                    .to_broadcast([128, 4, BLK]),
                    wc2rep, alu.mult,
                )

            # ================= phase 3: scan chunks =================
            tc.tile_set_cur_wait(2.0)
            for c in range(NCH):
                cc = c % CPB
                b = c // CPB
                qs = slice(c * Q, (c + 1) * Q)
                if cc == 0:
                    y2 = y2blk.tile([128, 8, BLK], f8, tag="y2")
                # xpos: PE-transpose xs chunk to position-major
                xposr = chkp.tile([128, 8, Q], bf16, tag="xposr")
                for w in range(2):
                    tp = pp_ytp.tile([128, 512], f32, tag="ytp")
                    tpb = tp.bitcast(bf16)
                    for ct in range(4):
                        nc.tensor.transpose(
                            tpb[:, ct * 128 : ct * 128 + 128],
                            xs_sb[:, w * 4 + ct, qs], ident_b,
                        )
                    nc.scalar.copy(xposr[:, w * 4 : w * 4 + 4, :], tpb[:, 0:512])
                xpv = xposr.rearrange("p t c -> p (t c)").rearrange(
                    "p (h c) -> p h c", c=P
                )
                # S psum + B transpose share one bank
                sps = pp_s.tile([128, 512], f32, tag="sps")
                nc.vector.memset(sps[:, 0:256], 0.0)
                # B position-major (bf16 view of spare sps columns)
                bpp = sps.bitcast(bf16)
                nc.tensor.transpose(bpp[:, 576:592], bct[0:16, qs], ident_b[0:16, 0:16])
                bpos = chkp.tile([128, NST], bf16, tag="bpos")
                nc.vector.tensor_copy(bpos, bpp[:, 576:592])
                # head-shared C@B^T -> G0^T[j, i], masked below the diagonal
                nc.tensor.matmul(
                    sps[:, 384:512], bct[0:16, qs], ct4[0:16, qs],
                    start=True, stop=True,
                )
                g0m = chkp.tile([128, Q], bf16, tag="g0m")
                nc.vector.tensor_tensor(g0m, sps[:, 384:512], tril01, alu.mult)
                # gdiff[j,i] = l_i - l_j + ln dt_j via K=2 fp16 matmuls
                # (chunk-recentered rows from dscr3; the shift cancels).
                # exp overflows to +inf above the diagonal; min(.,100) then
                # the g0m mask zeroes those entries.
                nc.gpsimd.dma_start(
                    lsg.rearrange("p h q -> p (h q)")[0:1, :], dscr3[16:32, qs]
                )
                nc.gpsimd.dma_start(
                    rsg.rearrange("p h q -> p (h q)")[1:2, :], dscr3[0:16, qs]
                )
                gdec = gdecp.tile([128, NH, Q], bf16, tag="gdec")
                gsb = gsbp.tile([128, NH, Q], bf16, tag="gsb")
                for g in range(4):
                    gdp = pp_gd.tile([128, 512], f32, tag="gd")
                    for hh_ in range(4):
                        h = g * 4 + hh_
                        lhsT = bass.AP(
                            lsg.tensor, lsg.offset + h * Q,
                            [[NH * Q, 2], [1, Q]],
                        )
                        rhs = bass.AP(
                            rsg.tensor, rsg.offset + h * Q,
                            [[NH * Q, 2], [1, Q]],
                        )
                        nc.tensor.matmul(
                            gdp[:, hh_ * 128 : hh_ * 128 + 128], lhsT, rhs,
                            start=True, stop=True, tile_position=(0, 0),
                        )
                    hsl = slice(g * 4, g * 4 + 4)
                    nc.scalar.activation(gdec[:, hsl, :], gdp, AF.Exp)
                    nc.vector.scalar_tensor_tensor(
                        gsb[:, hsl, :], gdec[:, hsl, :], 100.0,
                        g0m.rearrange("p (one i) -> p one i", one=1)
                        .to_broadcast([128, 4, Q]),
                        alu.min, alu.mult,
                    )
                # wS column = gdec[:, :, last] = dt_j exp(T - l_j); B_ws
                bws = chkp.tile([128, NH, NST], bf16, tag="bws")
                nc.vector.tensor_tensor(
                    bws,
                    bpos.rearrange("p (one n) -> p one n", one=1).to_broadcast([128, NH, NST]),
                    gdec[:, :, 127:128].to_broadcast([128, NH, NST]),
                    alu.mult,
                )
                # Y psums: pairs 0-3 and 4-7
                yA = pp_ytp.tile([128, 512], f32, tag="ytp")
                yB = pp_ytp.tile([128, 512], f32, tag="ytp")
                ypair = [yA, yB]
                for h in range(NH):
                    k, t = h % 4, h // 4
                    # S^T = B_ws.T @ X_h
                    nc.tensor.matmul(
                        sps[32 * k : 32 * k + 16, t * 64 : t * 64 + 64],
                        bws[:, h, :], xpv[:, h, :],
                        start=True, stop=True, tile_position=(0, 32 * k),
                    )
                for h in range(NH):
                    k, t = h % 4, h // 4
                    pr = h // 2
                    # Y^T = X_h.T @ G^T (+ H^T.T @ C''^T)
                    yp = ypair[pr // 4]
                    ysl = (slice(64 * (h % 2), 64 * (h % 2) + 64),
                           slice((pr % 4) * 128, (pr % 4) * 128 + 128))
                    nc.tensor.matmul(
                        yp[ysl[0], ysl[1]], xpv[:, h, :], gsb[:, h, :],
                        start=True, stop=(c == 0),
                    )
                    if c > 0:
                        nc.tensor.matmul(
                            yp[ysl[0], ysl[1]],
                            st[c % 2][32 * k : 32 * k + 16, t, :],
                            c2t[32 * k : 32 * k + 16, t, qs],
                            start=False, stop=True,
                            tile_position=(32 * k, 64 * (h % 2)),
                        )
                # state recurrence
                for t in range(4):
                    nc.vector.scalar_tensor_tensor(
                        st[(c + 1) % 2][:, t, :], st[c % 2][:, t, :],
                        texp_st[:, t, c : c + 1], sps[:, t * 64 : t * 64 + 64],
                        alu.mult, alu.add,
                    )
                # ---------- gating (y8 = gated-y/8 in fp8) ----------
                y8 = y8p.tile([128, 8, Q], f8, tag="y8")
                tmp = gp2.tile([128, 4, Q], f32, tag="gtmp")
                for half in range(2):
                    yp = ypair[half]
                    hsl4 = slice(half * 4, half * 4 + 4)
                    for pr4 in range(4):
                        pr = half * 4 + pr4
                        nc.vector.scalar_tensor_tensor(
                            tmp[:, pr4, :], xs_sb[:, pr, qs],
                            hp_sb[:, 2 + pr : 3 + pr],
                            yp[:, pr4 * 128 : pr4 * 128 + 128],
                            alu.mult, alu.add,
                        )
                    nc.vector.scalar_tensor_tensor(
                        y8[:, hsl4, :], tmp, 0.125, zs[:, hsl4, qs],
                        alu.mult, alu.mult,
                    )
                    nc.vector.tensor_tensor(
                        y2[:, hsl4, cc * Q : cc * Q + Q], y8[:, hsl4, :],
                        y8[:, hsl4, :], alu.mult,
                    )
                # ---------- U matmuls (fp8 DoubleRow) + store ----------
                for oc in range(2):
                    ups = pp_mm.tile([128, BLK], f32, tag="mm")
                    for cj in range(4):
                        nc.tensor.matmul(
                            ups, y8[:, 2 * cj : 2 * cj + 2, :],
                            mft_sb[:, 2 * cj : 2 * cj + 2, oc * 512 : oc * 512 + 512],
                            start=(cj == 0), stop=(cj == 3), perf_mode=DR,
                        )
                    usb = y2p.tile([128, BLK], bf16, tag="usb")
                    nc.scalar.copy(usb, ups)
                    nc.gpsimd.dma_start(
                        u[c * 128 : c * 128 + 128, oc * 512 : oc * 512 + 512], usb
                    )
                # block sumsq
                if cc == CPB - 1:
                    ssps = pp_mm.tile([128, BLK], f32, tag="mm")
                    for ct in range(8):
                        nc.tensor.matmul(
                            ssps[0:1, :], onescol, y2[:, ct, :],
                            start=(ct == 0), stop=(ct == 7),
                        )
                    ssb = y2p.tile([1, BLK], f32, tag="ssb")
                    nc.vector.tensor_copy(ssb, ssps[0:1, :])
                    nc.sync.dma_start(s[0:1, b * BLK : (b + 1) * BLK], ssb)

    nc.finalize()
    return nc


def _get_nc():
    if "nc" not in _cache:
        _cache["nc"] = _build_nc()
    return _cache["nc"]


def _prep_core_inputs(inputs, b, d, hh):
    import ml_dtypes

    fp8 = ml_dtypes.float8_e4m3

    def to8(a):
        return np.clip(a, -240.0, 240.0).astype(fp8)

    pre = "fwd" if d == 0 else "bwd"
    W = np.asarray(inputs[f"{pre}_in_proj_w"], np.float32)  # (4160, 1024)
    x = np.asarray(inputs["x"], np.float32)[b]  # (L, 1024)
    if d == 1:
        x = x[::-1]
    # x^T as (128, 8, L)
    xtv = np.ascontiguousarray(x.T.reshape(8, 128, L).transpose(1, 0, 2))
    # in_proj^T columns: [xs 1024 | B 16 | C 16 | dt 16 | z 1024], x16 for fp8
    W_xs = W[2048 + hh * 1024 : 3072 + hh * 1024]
    W_B = W[4096:4112]
    W_C = W[4112:4128]
    W_dt = W[4128 + hh * 16 : 4144 + hh * 16]
    W_z = W[hh * 1024 : 1024 + hh * 1024]
    Wt = np.concatenate([W_xs, W_B, W_C, W_dt, W_z], axis=0).T * 16.0  # (1024, 2096)
    wtv = np.ascontiguousarray(Wt.reshape(8, 128, 2096).transpose(1, 0, 2))
    # fused output projection, x64 for fp8; y8 carries 1/8 -> U is 8x
    Wo = np.asarray(inputs[f"{pre}_out_proj_w"], np.float32)  # (1024, 2048)
    Wl = np.asarray(inputs["layer_out_proj_w"], np.float32)  # (1024, 2048)
    nw = np.asarray(inputs[f"{pre}_norm_w"], np.float32)
    ch = slice(hh * 1024, hh * 1024 + 1024)
    M = (Wl[:, d * 1024 : d * 1024 + 1024] @ Wo)[:, ch] * nw[ch][None, :]
    MfT = M.T * 64.0  # (1024 c, 1024 o)
    mftv = np.ascontiguousarray(MfT.reshape(8, 128, 1024).transpose(1, 0, 2))
    # conv: diag values per (c-tile, tap) column
    cwf = np.asarray(inputs[f"{pre}_conv_w"], np.float32)[:, 0, :]  # (2080, 4)
    cwl = np.concatenate([cwf[hh * 1024 : 1024 + hh * 1024], cwf[2048:2080]], axis=0)
    cwv = np.zeros((128, 36), np.float32)
    for ct in range(9):
        n = 128 if ct < 8 else 32
        for k in range(4):
            cwv[:n, ct * 4 + k] = cwl[ct * 128 : ct * 128 + n, k]
    cbf = np.asarray(inputs[f"{pre}_conv_b"], np.float32)
    cbl = np.concatenate([cbf[hh * 1024 : 1024 + hh * 1024], cbf[2048:2080]])
    cbv = np.zeros((128, 18), np.float32)
    for ct in range(9):
        n = 128 if ct < 8 else 32
        cbv[:n, ct] = cbl[ct * 128 : ct * 128 + n]
    # host params
    hpv = np.zeros((128, 18), np.float32)
    hs = slice(hh * 16, hh * 16 + 16)
    hpv[32:48, 0] = np.asarray(inputs[f"{pre}_dt_bias"], np.float32)[hs]
    hpv[32:48, 1] = -np.exp(np.asarray(inputs[f"{pre}_A_log"], np.float32)[hs])
    Dp = np.asarray(inputs[f"{pre}_Dp"], np.float32)[hs]
    for h in range(NH):
        hpv[:, 2 + h] = Dp[h]
    return {
        "xt": to8(xtv),
        "wt": to8(wtv),
        "mft": to8(mftv),
        "cw": cwv,
        "cb": cbv,
        "hp": hpv,
    }


def _combine(inputs, results):
    x = np.asarray(inputs["x"], np.float32)
    scale = np.asarray(inputs["layer_scale"], np.float32)
    out = x.copy()
    i = 0
    for b in range(2):
        for d in range(2):
            U0 = np.asarray(results[i]["u"], np.float32)
            s0 = results[i]["s"][0]
            U1 = np.asarray(results[i + 1]["u"], np.float32)
            s1 = results[i + 1]["s"][0]
            i += 2
            r = 1.0 / np.sqrt((s0 + s1) / 32.0 + EPS)
            contrib = r[:, None] * (U0 + U1) / 8.0
            if d == 1:
                contrib = contrib[::-1]
            out[b] += contrib * scale[None, :]
    return out


def _run(inputs, trace=False, core_ids=None):
    from concourse.bass_utils import run_bass_kernel_spmd

    nc = _get_nc()
    in_maps = []
    for b in range(2):
        for d in range(2):
            for hh in range(2):
                in_maps.append(_prep_core_inputs(inputs, b, d, hh))
    if core_ids is None:
        core_ids = list(range(8))
    res = run_bass_kernel_spmd(
        nc, in_maps[: len(core_ids)], core_ids=core_ids, trace=trace
    )
    return res


def kernel(**inputs):
    res = _run(inputs)
    return _combine(inputs, res.results)


# revision 47
# speedup vs baseline: 1.0835x; 1.0835x over previous
"""BiMambaLayer Trainium2 kernel.

Sharding: 8 cores = batch(2) x direction(2) x head-half(2). Each core runs the
full L=2048 sequence of one (batch, direction) through 16 of the 32 heads of
that direction's Mamba2 block, plus the fused output projection restricted to
its 1024 d_inner channels. The gated-RMSNorm row scale commutes with the output
projections, so each core returns an unnormalized partial projection U and a
partial sum-of-squares s; the host combines:
    out[b] = x[b] + scale * sum_dir flip_d( r_d[:,None] * (U0 + U1) / 8 ),
    r_d = rsqrt((s0 + s1)/32 + eps).

Three phases over the whole sequence (so the activation table never thrashes
between the exp/ln set and the silu set, and the PE stays warm through dense
GEMM bursts):
  1) in_proj (fp8 DoubleRow) + dt pipeline for all 4 blocks of 512,
  2) z-proj (fp8 DoubleRow) + causal conv (fp8 diag matmuls) + all silus,
  3) 16 scan chunks of 128: head-shared C@B^T, per-head decay
     exp(l_i - l_j + ln dt_j) with the tri mask applied as min(exp,100)*g0m
     (masked entries overflow exp to +inf; min() tames them before the 0
     mask), Y^T = X^T.T@G^T + D_h*X^T + H^T.T@C''^T in PSUM, chunk-state
     recurrence on DVE, gating y8 = y*silu(z)/8 in fp8, U = y8 @ Mfused
     (fp8 DoubleRow).
fp8 scaling: in_proj weights x16 (undone at PSUM read), Mfused x64, y x1/8;
the host combine divides U by 8 and rescales s accordingly.
"""
import numpy as np

L = 2048
DM = 1024  # d_model
Q = 128  # scan chunk
NCH = L // Q  # 16 chunks
BLK = 512
NBLK = L // BLK  # 4
CPB = BLK // Q  # 4 chunks per block
NH = 16  # local heads
P = 64  # head dim
NST = 16  # state dim
EPS = 1e-5

_cache = {}


def _build_nc():
    import concourse.bass as bass
    import concourse.tile as tile
    import concourse.mybir as mybir
    from concourse import bacc
    from concourse.masks import make_identity
    from concourse.alu_op_type import AluOpType as alu

    f32 = mybir.dt.float32
    f16 = mybir.dt.float16
    bf16 = mybir.dt.bfloat16
    f8 = mybir.dt.float8e4
    AF = mybir.ActivationFunctionType
    DR = mybir.MatmulPerfMode.DoubleRow

    nc = bacc.Bacc(trn_type="TRN2")

    # ---- DRAM I/O (per-core shapes; host prepares layouts) ----
    xt = nc.dram_tensor("xt", [128, 8, L], f8, kind="ExternalInput")
    wt = nc.dram_tensor("wt", [128, 8, 2096], f8, kind="ExternalInput")
    mft = nc.dram_tensor("mft", [128, 8, DM], f8, kind="ExternalInput")
    cw = nc.dram_tensor("cw", [128, 36], f32, kind="ExternalInput")
    cb = nc.dram_tensor("cb", [128, 18], f32, kind="ExternalInput")
    hp = nc.dram_tensor("hp", [128, 18], f32, kind="ExternalInput")
    u = nc.dram_tensor("u", [L, DM], bf16, kind="ExternalOutput")
    s = nc.dram_tensor("s", [1, L], f32, kind="ExternalOutput")

    from contextlib import ExitStack

    with tile.TileContext(nc) as tc, ExitStack() as ctx:
        ep = ctx.enter_context
        const = ep(tc.tile_pool(name="const", bufs=1))
        seqp = ep(tc.tile_pool(name="seqp", bufs=1))
        statep = ep(tc.tile_pool(name="state", bufs=1))
        xtp = ep(tc.tile_pool(name="xtp", bufs=2))
        dtlp = ep(tc.tile_pool(name="dtlp", bufs=2))
        wcrp = ep(tc.tile_pool(name="wcrp", bufs=1))
        dscrp = ep(tc.tile_pool(name="dscrp", bufs=1, space="DRAM"))
        gp2 = ep(tc.tile_pool(name="gp2", bufs=2))
        chkp = ep(tc.tile_pool(name="chkp", bufs=3))
        gdecp = ep(tc.tile_pool(name="gdecp", bufs=3))
        gsbp = ep(tc.tile_pool(name="gsbp", bufs=3))
        y2p = ep(tc.tile_pool(name="y2p", bufs=2))
        y8p = ep(tc.tile_pool(name="y8p", bufs=2))
        y2blk = ep(tc.tile_pool(name="y2blk", bufs=1))
        pp_mm = ep(tc.tile_pool(name="pp_mm", bufs=2, space="PSUM"))
        pp_ytp = ep(tc.tile_pool(name="pp_ytp", bufs=3, space="PSUM"))
        pp_s = ep(tc.tile_pool(name="pp_s", bufs=2, space="PSUM"))
        pp_gd = ep(tc.tile_pool(name="pp_gd", bufs=1, space="PSUM"))
        if True:
            # ---------- constants / persistent ----------
            wt_sb = const.tile([128, 8, 2096], f8)
            nc.sync.dma_start(wt_sb, wt[:, :, :])
            mft_sb = const.tile([128, 8, DM], f8)
            nc.sync.dma_start(mft_sb, mft[:, :, :])
            cw_sb = const.tile([128, 36], f32)
            nc.sync.dma_start(cw_sb, cw[:, :])
            cb_sb = const.tile([128, 18], f32)
            nc.sync.dma_start(cb_sb, cb[:, :])
            hp_sb = const.tile([128, 18], f32)
            nc.sync.dma_start(hp_sb, hp[:, :])

            ident_b = const.tile([128, 128], bf16)
            make_identity(nc, ident_b)
            ident_f = const.tile([128, 128], f32)
            make_identity(nc, ident_f)
            # multiplicative mask, [j, i] coords: 1 where i >= j, 0 where i < j
            tril01 = const.tile([128, 128], bf16)
            nc.gpsimd.memset(tril01, 1.0)
            nc.gpsimd.affine_select(
                out=tril01, in_=tril01, compare_op=mybir.AluOpType.is_ge,
                fill=0.0, base=0, pattern=[[1, 128]], channel_multiplier=-1,
            )
            onesq = const.tile([128, 128], f32)
            nc.vector.memset(onesq, 1.0)
            onescol = const.tile([128, 1], f8)
            nc.vector.memset(onescol, 1.0)
            # conv diagonal weight tiles (fp8), built from cw columns
            convd = const.tile([128, 36, 128], f8)
            for j in range(36):
                nc.scalar.mul(convd[:, j, :], ident_b, cw_sb[:, j : j + 1])
            # persistent scan state: 4 head-groups (heads 4t+k at partitions
            # 32k..32k+16), ping-pong A/B
            stA = statep.tile([128, 4, P], f8, tag="stA")
            stB = statep.tile([128, 4, P], f8, tag="stB")
            nc.vector.memset(stA, 0.0)
            nc.vector.memset(stB, 0.0)
            st = [stA, stB]
            # chunk-decay per-partition scalars, [state-tile, chunk]
            texp_st = statep.tile([128, 4, NCH], f32, tag="texp")
            nc.vector.memset(texp_st, 0.0)

            # whole-sequence activations
            xsr = seqp.tile([128, 8, L + 3], f8)  # in_proj xs (+3 zero halo)
            bcr = seqp.tile([32, L + 3], f8)
            xs_sb = seqp.tile([128, 8, L], bf16)  # conv+silu out
            bct = seqp.tile([32, L], bf16)
            ct4 = seqp.tile([128, L], bf16)  # C rows replicated to 4 bases
            zs = seqp.tile([128, 8, L], f8)  # silu(z)
            c2t = seqp.tile([128, 4, L], f8)  # C * exp(l_h) per head
            nc.vector.memset(xsr[:, :, 0:3], 0.0)
            nc.vector.memset(bcr[:, 0:3], 0.0)
            # gdiff matmul operands (fp16, chunk-recentered l so fp16 is
            # accurate): partition 0/1 = {per-chunk data, constant}
            lsg = statep.tile([2, NH, Q], f16, tag="lsg")  # p0: l-lndt-T, p1: 1
            rsg = statep.tile([2, NH, Q], f16, tag="rsg")  # p0: -1, p1: l-T
            nc.vector.memset(lsg, 1.0)
            nc.vector.memset(rsg, -1.0)

            dscr = dscrp.tile([48, L], f32, tag="dscr")
            dscr2 = dscrp.tile([16, L], bf16, tag="dscr2")
            dscr3 = dscrp.tile([32, L], f16, tag="dscr3")

            # ================= phase 1: in_proj + dt =================
            for b in range(NBLK):
                bsl = slice(b * BLK, (b + 1) * BLK)
                bsl3 = slice(3 + b * BLK, 3 + (b + 1) * BLK)
                xtb = xtp.tile([128, 8, BLK], f8, tag="xtb")
                nc.sync.dma_start(xtb, xt[:, :, bsl])
                dt_sp = dtlp.tile([128, BLK], f32, tag="dtsp")
                for et in range(9):
                    m = 128 if et < 8 else 48
                    ecol = et * 128 if et < 8 else 1024
                    ps = [pp_mm, pp_ytp][et % 2].tile(
                        [128, BLK], f32, tag=["mm", "ytp"][et % 2])
                    for kj in range(4):
                        nc.tensor.matmul(
                            ps[:m, :], wt_sb[:, 2 * kj : 2 * kj + 2, ecol : ecol + m],
                            xtb[:, 2 * kj : 2 * kj + 2, :],
                            start=(kj == 0), stop=(kj == 3), perf_mode=DR,
                        )
                    if et < 8:
                        nc.scalar.mul(xsr[:, et, bsl3], ps, 0.0625)
                    else:
                        nc.scalar.mul(bcr[:, bsl3], ps[0:32, :], 0.0625)
                        nc.scalar.activation(
                            dt_sp[32:48, :], ps[32:48, :], AF.Exp,
                            bias=hp_sb[32:48, 0:1], scale=0.0625,
                        )
                        nc.vector.tensor_scalar_add(
                            dt_sp[32:48, :], dt_sp[32:48, :], 1.0
                        )
                        nc.scalar.activation(dt_sp[32:48, :], dt_sp[32:48, :], AF.Ln)
                # ---------- dt pipeline ----------
                lndt = dtlp.tile([128, BLK], f32, tag="lndt")
                lcm = dtlp.tile([128, BLK], f32, tag="lcm")
                wc2 = dtlp.tile([128, BLK], bf16, tag="wc2")
                nc.scalar.activation(lndt[32:48, :], dt_sp[32:48, :], AF.Ln)
                dtA = dtlp.tile([128, BLK], f32, tag="dtA")
                nc.vector.tensor_scalar_mul(
                    dtA[32:48, :], dt_sp[32:48, :], hp_sb[32:48, 1:2]
                )
                for cc in range(CPB):
                    qs = slice(cc * Q, (cc + 1) * Q)
                    nc.vector.tensor_tensor_scan(
                        lcm[32:48, qs], onesq[32:48, :], dtA[32:48, qs],
                        0.0, alu.mult, alu.add,
                    )
                nc.scalar.activation(wc2[32:48, :], lcm[32:48, :], AF.Exp)
                texp_cm = dtlp.tile([128, CPB, 1], f32, tag="texpcm")
                lv = lcm[32:48, :].rearrange("p (c q) -> p c q", q=Q)
                nc.scalar.activation(texp_cm[32:48, :, :], lv[:, :, 127:128], AF.Exp)
                # chunk-recentered l rows in fp16 for the gdiff matmuls
                lcmr = dtlp.tile([128, BLK], f16, tag="lcmr")
                lsubr = dtlp.tile([128, BLK], f16, tag="lsubr")
                nc.vector.tensor_tensor(
                    lcmr[32:48, :].rearrange("p (c q) -> p c q", q=Q),
                    lv, lv[:, :, 127:128].to_broadcast([16, CPB, Q]),
                    alu.subtract,
                )
                nc.vector.tensor_tensor(
                    lsubr[32:48, :], lcmr[32:48, :], lndt[32:48, :], alu.subtract
                )
                # bounce small per-block vectors through DRAM so they can be
                # partition-broadcast on the way back in
                nc.sync.dma_start(dscr3[0:16, bsl], lcmr[32:48, :])
                nc.sync.dma_start(dscr3[16:32, bsl], lsubr[32:48, :])
                nc.sync.dma_start(dscr2[:, bsl], wc2[32:48, :])
                nc.sync.dma_start(
                    dscr[32:48, b * CPB : (b + 1) * CPB],
                    texp_cm[32:48, :, :].rearrange("p c one -> p (c one)"),
                )
                for k in range(4):
                    nc.sync.dma_start(
                        texp_st[32 * k : 32 * k + 16, :, b * CPB : (b + 1) * CPB],
                        bass.AP(dscr.tensor,
                                dscr.offset + (32 + k) * L + b * CPB,
                                [[0, 16], [4 * L, 4], [1, CPB]]),
                    )

            # ================= phase 2: z + conv (all silus) =================
            tc.tile_set_cur_wait(1.0)
            for b in range(NBLK):
                bsl = slice(b * BLK, (b + 1) * BLK)
                xtb = xtp.tile([128, 8, BLK], f8, tag="xtb")
                nc.sync.dma_start(xtb, xt[:, :, bsl])
                for zt in range(8):
                    ps = [pp_mm, pp_ytp][zt % 2].tile(
                        [128, BLK], f32, tag=["mm", "ytp"][zt % 2])
                    for kj in range(4):
                        nc.tensor.matmul(
                            ps, wt_sb[:, 2 * kj : 2 * kj + 2, 1072 + zt * 128 : 1200 + zt * 128],
                            xtb[:, 2 * kj : 2 * kj + 2, :],
                            start=(kj == 0), stop=(kj == 3), perf_mode=DR,
                        )
                    nc.scalar.activation(zs[:, zt, bsl], ps, AF.Silu, scale=0.0625)
                for ct in range(9):
                    m = 128 if ct < 8 else 32
                    ps = [pp_mm, pp_ytp][ct % 2].tile(
                        [128, BLK], f32, tag=["mm", "ytp"][ct % 2])
                    for k in range(4):
                        a = b * BLK + k
                        mov = (xsr[:, ct, a : a + BLK] if ct < 8
                               else bcr[:, a : a + BLK])
                        nc.tensor.matmul(
                            ps[:m, :], convd[:m, ct * 4 + k, :m], mov,
                            start=(k == 0), stop=(k == 3),
                        )
                    dst = xs_sb[:, ct, bsl] if ct < 8 else bct[:, bsl]
                    nc.scalar.activation(
                        dst, ps[:m, :], AF.Silu, bias=cb_sb[:m, ct : ct + 1]
                    )
                # C rows replicated to the four 32-aligned bases
                for k4 in range(4):
                    nc.sync.dma_start(ct4[32 * k4 : 32 * k4 + 16, bsl], bct[16:32, bsl])
                # C'' = C * exp(l_h) per head
                wc2rep = wcrp.tile([128, 4, BLK], bf16, tag="wc2rep")
                _qw = [nc.scalar, nc.gpsimd, nc.sync, nc.gpsimd]
                for k in range(4):
                    _qw[k].dma_start(
                        wc2rep[32 * k : 32 * k + 16, :, :],
                        bass.AP(dscr2.tensor, dscr2.offset + k * L + b * BLK,
                                [[0, 16], [4 * L, 4], [1, BLK]]),
                    )
                nc.vector.tensor_tensor(
                    c2t[:, :, bsl],
                    ct4[:, bsl].rearrange("p (one c) -> p one c", one=1)
                    .to_broadcast([128, 4, BLK]),
                    wc2rep, alu.mult,
                )

            # ================= phase 3: scan chunks =================
            tc.tile_set_cur_wait(2.0)
            for c in range(NCH):
                cc = c % CPB
                b = c // CPB
                qs = slice(c * Q, (c + 1) * Q)
                if cc == 0:
                    y2 = y2blk.tile([128, 8, BLK], f8, tag="y2")
                # xpos: PE-transpose xs chunk to position-major
                xposr = chkp.tile([128, 8, Q], bf16, tag="xposr")
                for w in range(2):
                    tp = pp_ytp.tile([128, 512], f32, tag="ytp")
                    tpb = tp.bitcast(bf16)
                    for ct in range(4):
                        nc.tensor.transpose(
                            tpb[:, ct * 128 : ct * 128 + 128],
                            xs_sb[:, w * 4 + ct, qs], ident_b,
                        )
                    nc.scalar.copy(xposr[:, w * 4 : w * 4 + 4, :], tpb[:, 0:512])
                xpv = xposr.rearrange("p t c -> p (t c)").rearrange(
                    "p (h c) -> p h c", c=P
                )
                # S psum + B transpose share one bank
                sps = pp_s.tile([128, 512], f32, tag="sps")
                nc.vector.memset(sps[:, 0:256], 0.0)
                # B position-major (bf16 view of spare sps columns)
                bpp = sps.bitcast(bf16)
                nc.tensor.transpose(bpp[:, 576:592], bct[0:16, qs], ident_b[0:16, 0:16])
                bpos = chkp.tile([128, NST], bf16, tag="bpos")
                nc.vector.tensor_copy(bpos, bpp[:, 576:592])
                # head-shared C@B^T -> G0^T[j, i], masked below the diagonal
                nc.tensor.matmul(
                    sps[:, 384:512], bct[0:16, qs], ct4[0:16, qs],
                    start=True, stop=True,
                )
                g0m = chkp.tile([128, Q], bf16, tag="g0m")
                nc.vector.tensor_tensor(g0m, sps[:, 384:512], tril01, alu.mult)
                # gdiff[j,i] = l_i - l_j + ln dt_j via K=2 fp16 matmuls
                # (chunk-recentered rows from dscr3; the shift cancels).
                # exp overflows to +inf above the diagonal; min(.,100) then
                # the g0m mask zeroes those entries.
                nc.gpsimd.dma_start(
                    lsg.rearrange("p h q -> p (h q)")[0:1, :], dscr3[16:32, qs]
                )
                nc.gpsimd.dma_start(
                    rsg.rearrange("p h q -> p (h q)")[1:2, :], dscr3[0:16, qs]
                )
                gdec = gdecp.tile([128, NH, Q], bf16, tag="gdec")
                gsb = gsbp.tile([128, NH, Q], bf16, tag="gsb")
                for g in range(4):
                    gdp = pp_gd.tile([128, 512], f32, tag="gd")
                    for hh_ in range(4):
                        h = g * 4 + hh_
                        lhsT = bass.AP(
                            lsg.tensor, lsg.offset + h * Q,
                            [[NH * Q, 2], [1, Q]],
                        )
                        rhs = bass.AP(
                            rsg.tensor, rsg.offset + h * Q,
                            [[NH * Q, 2], [1, Q]],
                        )
                        nc.tensor.matmul(
                            gdp[:, hh_ * 128 : hh_ * 128 + 128], lhsT, rhs,
                            start=True, stop=True, tile_position=(0, 0),
                        )
                    hsl = slice(g * 4, g * 4 + 4)
                    nc.scalar.activation(gdec[:, hsl, :], gdp, AF.Exp)
                    nc.vector.scalar_tensor_tensor(
                        gsb[:, hsl, :], gdec[:, hsl, :], 100.0,
                        g0m.rearrange("p (one i) -> p one i", one=1)
                        .to_broadcast([128, 4, Q]),
                        alu.min, alu.mult,
                    )
                # wS column = gdec[:, :, last] = dt_j exp(T - l_j); B_ws
                bws = chkp.tile([128, NH, NST], bf16, tag="bws")
                nc.vector.tensor_tensor(
                    bws,
                    bpos.rearrange("p (one n) -> p one n", one=1).to_broadcast([128, NH, NST]),
                    gdec[:, :, 127:128].to_broadcast([128, NH, NST]),
                    alu.mult,
                )
                # Y psums: pairs 0-3 and 4-7
                yA = pp_ytp.tile([128, 512], f32, tag="ytp")
                yB = pp_ytp.tile([128, 512], f32, tag="ytp")
                ypair = [yA, yB]
                for h in range(NH):
                    k, t = h % 4, h // 4
                    # S^T = B_ws.T @ X_h
                    nc.tensor.matmul(
                        sps[32 * k : 32 * k + 16, t * 64 : t * 64 + 64],
                        bws[:, h, :], xpv[:, h, :],
                        start=True, stop=True, tile_position=(0, 32 * k),
                    )
                for h in range(NH):
                    k, t = h % 4, h // 4
                    pr = h // 2
                    # Y^T = X_h.T @ G^T (+ H^T.T @ C''^T)
                    yp = ypair[pr // 4]
                    ysl = (slice(64 * (h % 2), 64 * (h % 2) + 64),
                           slice((pr % 4) * 128, (pr % 4) * 128 + 128))
                    nc.tensor.matmul(
                        yp[ysl[0], ysl[1]], xpv[:, h, :], gsb[:, h, :],
                        start=True, stop=(c == 0),
                    )
                    if c > 0:
                        nc.tensor.matmul(
                            yp[ysl[0], ysl[1]],
                            st[c % 2][32 * k : 32 * k + 16, t, :],
                            c2t[32 * k : 32 * k + 16, t, qs],
                            start=False, stop=True,
                            tile_position=(32 * k, 64 * (h % 2)),
                        )
                # state recurrence
                for t in range(4):
                    nc.vector.scalar_tensor_tensor(
                        st[(c + 1) % 2][:, t, :], st[c % 2][:, t, :],
                        texp_st[:, t, c : c + 1], sps[:, t * 64 : t * 64 + 64],
                        alu.mult, alu.add,
                    )
                # ---------- gating (y8 = gated-y/8 in fp8) ----------
                y8 = y8p.tile([128, 8, Q], f8, tag="y8")
                tmp = gp2.tile([128, 4, Q], f32, tag="gtmp")
                for half in range(2):
                    yp = ypair[half]
                    hsl4 = slice(half * 4, half * 4 + 4)
                    for pr4 in range(4):
                        pr = half * 4 + pr4
                        nc.vector.scalar_tensor_tensor(
                            tmp[:, pr4, :], xs_sb[:, pr, qs],
                            hp_sb[:, 2 + pr : 3 + pr],
                            yp[:, pr4 * 128 : pr4 * 128 + 128],
                            alu.mult, alu.add,
                        )
                    nc.vector.scalar_tensor_tensor(
                        y8[:, hsl4, :], tmp, 0.125, zs[:, hsl4, qs],
                        alu.mult, alu.mult,
                    )
                    nc.vector.tensor_tensor(
                        y2[:, hsl4, cc * Q : cc * Q + Q], y8[:, hsl4, :],
                        y8[:, hsl4, :], alu.mult,
                    )
                # ---------- U matmuls (fp8 DoubleRow) + store ----------
                for oc in range(2):
                    ups = pp_mm.tile([128, BLK], f32, tag="mm")
                    for cj in range(4):
                        nc.tensor.matmul(
                            ups, y8[:, 2 * cj : 2 * cj + 2, :],
                            mft_sb[:, 2 * cj : 2 * cj + 2, oc * 512 : oc * 512 + 512],
                            start=(cj == 0), stop=(cj == 3), perf_mode=DR,
                        )
                    usb = y2p.tile([128, BLK], bf16, tag="usb")
                    nc.scalar.copy(usb, ups)
                    nc.gpsimd.dma_start(
                        u[c * 128 : c * 128 + 128, oc * 512 : oc * 512 + 512], usb
                    )
                # block sumsq
                if cc == CPB - 1:
                    ssps = pp_mm.tile([128, BLK], f32, tag="mm")
                    for ct in range(8):
                        nc.tensor.matmul(
                            ssps[0:1, :], onescol, y2[:, ct, :],
                            start=(ct == 0), stop=(ct == 7),
                        )
                    ssb = y2p.tile([1, BLK], f32, tag="ssb")
                    nc.vector.tensor_copy(ssb, ssps[0:1, :])
                    nc.sync.dma_start(s[0:1, b * BLK : (b + 1) * BLK], ssb)

    nc.finalize()
    return nc


def _get_nc():
    if "nc" not in _cache:
        _cache["nc"] = _build_nc()
    return _cache["nc"]


def _prep_core_inputs(inputs, b, d, hh):
    import ml_dtypes

    fp8 = ml_dtypes.float8_e4m3

    def to8(a):
        return np.clip(a, -240.0, 240.0).astype(fp8)

    pre = "fwd" if d == 0 else "bwd"
    W = np.asarray(inputs[f"{pre}_in_proj_w"], np.float32)  # (4160, 1024)
    x = np.asarray(inputs["x"], np.float32)[b]  # (L, 1024)
    if d == 1:
        x = x[::-1]
    # x^T as (128, 8, L)
    xtv = np.ascontiguousarray(x.T.reshape(8, 128, L).transpose(1, 0, 2))
    # in_proj^T columns: [xs 1024 | B 16 | C 16 | dt 16 | z 1024], x16 for fp8
    W_xs = W[2048 + hh * 1024 : 3072 + hh * 1024]
    W_B = W[4096:4112]
    W_C = W[4112:4128]
    W_dt = W[4128 + hh * 16 : 4144 + hh * 16]
    W_z = W[hh * 1024 : 1024 + hh * 1024]
    Wt = np.concatenate([W_xs, W_B, W_C, W_dt, W_z], axis=0).T * 16.0  # (1024, 2096)
    wtv = np.ascontiguousarray(Wt.reshape(8, 128, 2096).transpose(1, 0, 2))
    # fused output projection, x64 for fp8; y8 carries 1/8 -> U is 8x
    Wo = np.asarray(inputs[f"{pre}_out_proj_w"], np.float32)  # (1024, 2048)
    Wl = np.asarray(inputs["layer_out_proj_w"], np.float32)  # (1024, 2048)
    nw = np.asarray(inputs[f"{pre}_norm_w"], np.float32)
    ch = slice(hh * 1024, hh * 1024 + 1024)
    M = (Wl[:, d * 1024 : d * 1024 + 1024] @ Wo)[:, ch] * nw[ch][None, :]
    MfT = M.T * 64.0  # (1024 c, 1024 o)
    mftv = np.ascontiguousarray(MfT.reshape(8, 128, 1024).transpose(1, 0, 2))
    # conv: diag values per (c-tile, tap) column
    cwf = np.asarray(inputs[f"{pre}_conv_w"], np.float32)[:, 0, :]  # (2080, 4)
    cwl = np.concatenate([cwf[hh * 1024 : 1024 + hh * 1024], cwf[2048:2080]], axis=0)
    cwv = np.zeros((128, 36), np.float32)
    for ct in range(9):
        n = 128 if ct < 8 else 32
        for k in range(4):
            cwv[:n, ct * 4 + k] = cwl[ct * 128 : ct * 128 + n, k]
    cbf = np.asarray(inputs[f"{pre}_conv_b"], np.float32)
    cbl = np.concatenate([cbf[hh * 1024 : 1024 + hh * 1024], cbf[2048:2080]])
    cbv = np.zeros((128, 18), np.float32)
    for ct in range(9):
        n = 128 if ct < 8 else 32
        cbv[:n, ct] = cbl[ct * 128 : ct * 128 + n]
    # host params
    hpv = np.zeros((128, 18), np.float32)
    hs = slice(hh * 16, hh * 16 + 16)
    hpv[32:48, 0] = np.asarray(inputs[f"{pre}_dt_bias"], np.float32)[hs]
    hpv[32:48, 1] = -np.exp(np.asarray(inputs[f"{pre}_A_log"], np.float32)[hs])
    Dp = np.asarray(inputs[f"{pre}_Dp"], np.float32)[hs]
    for h in range(NH):
        hpv[:, 2 + h] = Dp[h]
    return {
        "xt": to8(xtv),
        "wt": to8(wtv),
        "mft": to8(mftv),
        "cw": cwv,
        "cb": cbv,
        "hp": hpv,
    }


def _combine(inputs, results):
    x = np.asarray(inputs["x"], np.float32)
    scale = np.asarray(inputs["layer_scale"], np.float32)
    out = x.copy()
    i = 0
    for b in range(2):
        for d in range(2):
            U0 = np.asarray(results[i]["u"], np.float32)
            s0 = results[i]["s"][0]
            U1 = np.asarray(results[i + 1]["u"], np.float32)
            s1 = results[i + 1]["s"][0]
            i += 2
            r = 1.0 / np.sqrt((s0 + s1) / 32.0 + EPS)
            contrib = r[:, None] * (U0 + U1) / 8.0
            if d == 1:
                contrib = contrib[::-1]
            out[b] += contrib * scale[None, :]
    return out


def _run(inputs, trace=False, core_ids=None):
    from concourse.bass_utils import run_bass_kernel_spmd

    nc = _get_nc()
    in_maps = []
    for b in range(2):
        for d in range(2):
            for hh in range(2):
                in_maps.append(_prep_core_inputs(inputs, b, d, hh))
    if core_ids is None:
        core_ids = list(range(8))
    res = run_bass_kernel_spmd(
        nc, in_maps[: len(core_ids)], core_ids=core_ids, trace=trace
    )
    return res


def kernel(**inputs):
    res = _run(inputs)
    return _combine(inputs, res.results)


# revision 49
# speedup vs baseline: 1.1803x; 1.0893x over previous
"""BiMambaLayer Trainium2 kernel.

Sharding: 8 cores = batch(2) x direction(2) x head-half(2). Each core runs the
full L=2048 sequence of one (batch, direction) through 16 of the 32 heads of
that direction's Mamba2 block, plus the fused output projection restricted to
its 1024 d_inner channels. The gated-RMSNorm row scale commutes with the output
projections, so each core returns an unnormalized partial projection U and a
partial sum-of-squares s; the host combines:
    out[b] = x[b] + scale * sum_dir flip_d( r_d[:,None] * (U0 + U1) / 8 ),
    r_d = rsqrt((s0 + s1)/32 + eps).

Three phases over the whole sequence (so the activation table never thrashes
between the exp/ln set and the silu set, and the PE stays warm through dense
GEMM bursts):
  1) in_proj (fp8 DoubleRow) + dt pipeline for all 4 blocks of 512,
  2) z-proj (fp8 DoubleRow) + causal conv (fp8 diag matmuls) + all silus,
  3) 16 scan chunks of 128: head-shared C@B^T, per-head decay
     exp(l_i - l_j + ln dt_j) with the tri mask applied as min(exp,100)*g0m
     (masked entries overflow exp to +inf; min() tames them before the 0
     mask), Y^T = X^T.T@G^T + D_h*X^T + H^T.T@C''^T in PSUM, chunk-state
     recurrence on DVE, gating y8 = y*silu(z)/8 in fp8, U = y8 @ Mfused
     (fp8 DoubleRow).
fp8 scaling: in_proj weights x16 (undone at PSUM read), Mfused x64, y x1/8;
the host combine divides U by 8 and rescales s accordingly.
"""
import numpy as np

L = 2048
DM = 1024  # d_model
Q = 128  # scan chunk
NCH = L // Q  # 16 chunks
BLK = 512
NBLK = L // BLK  # 4
CPB = BLK // Q  # 4 chunks per block
NH = 16  # local heads
P = 64  # head dim
NST = 16  # state dim
EPS = 1e-5

_cache = {}


def _build_nc():
    import concourse.bass as bass
    import concourse.tile as tile
    import concourse.mybir as mybir
    from concourse import bacc
    from concourse.masks import make_identity
    from concourse.alu_op_type import AluOpType as alu

    f32 = mybir.dt.float32
    f16 = mybir.dt.float16
    bf16 = mybir.dt.bfloat16
    f8 = mybir.dt.float8e4
    AF = mybir.ActivationFunctionType
    DR = mybir.MatmulPerfMode.DoubleRow

    nc = bacc.Bacc(trn_type="TRN2")

    # ---- DRAM I/O (per-core shapes; host prepares layouts) ----
    xt = nc.dram_tensor("xt", [128, 8, L], f8, kind="ExternalInput")
    wt = nc.dram_tensor("wt", [128, 8, 2096], f8, kind="ExternalInput")
    mft = nc.dram_tensor("mft", [128, 8, DM], f8, kind="ExternalInput")
    cw = nc.dram_tensor("cw", [128, 36], f32, kind="ExternalInput")
    cb = nc.dram_tensor("cb", [128, 18], f32, kind="ExternalInput")
    hp = nc.dram_tensor("hp", [128, 18], f32, kind="ExternalInput")
    u = nc.dram_tensor("u", [L, DM], bf16, kind="ExternalOutput")
    s = nc.dram_tensor("s", [1, L], f32, kind="ExternalOutput")

    from contextlib import ExitStack

    with tile.TileContext(nc) as tc, ExitStack() as ctx:
        ep = ctx.enter_context
        const = ep(tc.tile_pool(name="const", bufs=1))
        seqp = ep(tc.tile_pool(name="seqp", bufs=1))
        statep = ep(tc.tile_pool(name="state", bufs=1))
        xtp = ep(tc.tile_pool(name="xtp", bufs=2))
        dtlp = ep(tc.tile_pool(name="dtlp", bufs=2))
        wcrp = ep(tc.tile_pool(name="wcrp", bufs=1))
        dscrp = ep(tc.tile_pool(name="dscrp", bufs=1, space="DRAM"))
        gp2 = ep(tc.tile_pool(name="gp2", bufs=2))
        lsgp = ep(tc.tile_pool(name="lsgp", bufs=2))
        chkp = ep(tc.tile_pool(name="chkp", bufs=3))
        gdecp = ep(tc.tile_pool(name="gdecp", bufs=2))
        gsbp = ep(tc.tile_pool(name="gsbp", bufs=2))
        y2p = ep(tc.tile_pool(name="y2p", bufs=2))
        y8p = ep(tc.tile_pool(name="y8p", bufs=2))
        y2blk = ep(tc.tile_pool(name="y2blk", bufs=1))
        pp_mm = ep(tc.tile_pool(name="pp_mm", bufs=2, space="PSUM"))
        pp_ytp = ep(tc.tile_pool(name="pp_ytp", bufs=3, space="PSUM"))
        pp_s = ep(tc.tile_pool(name="pp_s", bufs=2, space="PSUM"))
        pp_gd = ep(tc.tile_pool(name="pp_gd", bufs=1, space="PSUM"))
        if True:
            # ---------- constants / persistent ----------
            wt_sb = const.tile([128, 8, 2096], f8)
            nc.sync.dma_start(wt_sb, wt[:, :, :])
            mft_sb = const.tile([128, 8, DM], f8)
            nc.sync.dma_start(mft_sb, mft[:, :, :])
            cw_sb = const.tile([128, 36], f32)
            nc.sync.dma_start(cw_sb, cw[:, :])
            cb_sb = const.tile([128, 18], f32)
            nc.sync.dma_start(cb_sb, cb[:, :])
            hp_sb = const.tile([128, 18], f32)
            nc.sync.dma_start(hp_sb, hp[:, :])

            ident_b = const.tile([128, 128], bf16)
            make_identity(nc, ident_b)
            ident_f = const.tile([128, 128], f32)
            make_identity(nc, ident_f)
            # multiplicative mask, [j, i] coords: 1 where i >= j, 0 where i < j
            tril01 = const.tile([128, 128], bf16)
            nc.gpsimd.memset(tril01, 1.0)
            nc.gpsimd.affine_select(
                out=tril01, in_=tril01, compare_op=mybir.AluOpType.is_ge,
                fill=0.0, base=0, pattern=[[1, 128]], channel_multiplier=-1,
            )
            onesq = const.tile([128, 128], f32)
            nc.vector.memset(onesq, 1.0)
            onescol = const.tile([128, 1], f8)
            nc.vector.memset(onescol, 1.0)
            # conv diagonal weight tiles (fp8), built from cw columns
            convd = const.tile([128, 36, 128], f8)
            for j in range(36):
                nc.scalar.mul(convd[:, j, :], ident_b, cw_sb[:, j : j + 1])
            # persistent scan state: 4 head-groups (heads 4t+k at partitions
            # 32k..32k+16), ping-pong A/B
            stA = statep.tile([128, 4, P], f8, tag="stA")
            stB = statep.tile([128, 4, P], f8, tag="stB")
            nc.vector.memset(stA, 0.0)
            nc.vector.memset(stB, 0.0)
            st = [stA, stB]
            # chunk-decay per-partition scalars, [state-tile, chunk]
            texp_st = statep.tile([128, 4, NCH], f32, tag="texp")
            nc.vector.memset(texp_st, 0.0)

            # whole-sequence activations
            xsr = seqp.tile([128, 8, L + 3], f8)  # in_proj xs (+3 zero halo)
            bcr = seqp.tile([32, L + 3], f8)
            xs_sb = seqp.tile([128, 8, L], bf16)  # conv+silu out
            bct = seqp.tile([32, L], bf16)
            ct4 = seqp.tile([128, L], bf16)  # C rows replicated to 4 bases
            zs = seqp.tile([128, 8, L], f8)  # silu(z)
            c2t = seqp.tile([128, 4, L], f8)  # C * exp(l_h) per head
            nc.vector.memset(xsr[:, :, 0:3], 0.0)
            nc.vector.memset(bcr[:, 0:3], 0.0)

            dscr = dscrp.tile([48, L], f32, tag="dscr")
            dscr2 = dscrp.tile([16, L], bf16, tag="dscr2")
            dscr3 = dscrp.tile([32, L], f16, tag="dscr3")

            # ================= phase 1: in_proj + dt =================
            for b in range(NBLK):
                bsl = slice(b * BLK, (b + 1) * BLK)
                bsl3 = slice(3 + b * BLK, 3 + (b + 1) * BLK)
                xtb = xtp.tile([128, 8, BLK], f8, tag="xtb")
                nc.sync.dma_start(xtb, xt[:, :, bsl])
                dt_sp = dtlp.tile([128, BLK], f32, tag="dtsp")
                for et in range(9):
                    m = 128 if et < 8 else 48
                    ecol = et * 128 if et < 8 else 1024
                    ps = [pp_mm, pp_ytp][et % 2].tile(
                        [128, BLK], f32, tag=["mm", "ytp"][et % 2])
                    for kj in range(4):
                        nc.tensor.matmul(
                            ps[:m, :], wt_sb[:, 2 * kj : 2 * kj + 2, ecol : ecol + m],
                            xtb[:, 2 * kj : 2 * kj + 2, :],
                            start=(kj == 0), stop=(kj == 3), perf_mode=DR,
                        )
                    if et < 8:
                        if et % 2 == 0:
                            nc.scalar.mul(xsr[:, et, bsl3], ps, 0.0625)
                        else:
                            nc.vector.tensor_scalar_mul(
                                xsr[:, et, bsl3], ps, 0.0625)
                    else:
                        nc.scalar.mul(bcr[:, bsl3], ps[0:32, :], 0.0625)
                        nc.scalar.activation(
                            dt_sp[32:48, :], ps[32:48, :], AF.Exp,
                            bias=hp_sb[32:48, 0:1], scale=0.0625,
                        )
                        nc.vector.tensor_scalar_add(
                            dt_sp[32:48, :], dt_sp[32:48, :], 1.0
                        )
                        nc.scalar.activation(dt_sp[32:48, :], dt_sp[32:48, :], AF.Ln)
                # ---------- dt pipeline ----------
                lndt = dtlp.tile([128, BLK], f32, tag="lndt")
                lcm = dtlp.tile([128, BLK], f32, tag="lcm")
                wc2 = dtlp.tile([128, BLK], bf16, tag="wc2")
                nc.scalar.activation(lndt[32:48, :], dt_sp[32:48, :], AF.Ln)
                dtA = dtlp.tile([128, BLK], f32, tag="dtA")
                nc.vector.tensor_scalar_mul(
                    dtA[32:48, :], dt_sp[32:48, :], hp_sb[32:48, 1:2]
                )
                for cc in range(CPB):
                    qs = slice(cc * Q, (cc + 1) * Q)
                    nc.vector.tensor_tensor_scan(
                        lcm[32:48, qs], onesq[32:48, :], dtA[32:48, qs],
                        0.0, alu.mult, alu.add,
                    )
                nc.scalar.activation(wc2[32:48, :], lcm[32:48, :], AF.Exp)
                texp_cm = dtlp.tile([128, CPB, 1], f32, tag="texpcm")
                lv = lcm[32:48, :].rearrange("p (c q) -> p c q", q=Q)
                nc.scalar.activation(texp_cm[32:48, :, :], lv[:, :, 127:128], AF.Exp)
                # chunk-recentered l rows in fp16 for the gdiff matmuls
                lcmr = dtlp.tile([128, BLK], f16, tag="lcmr")
                lsubr = dtlp.tile([128, BLK], f16, tag="lsubr")
                nc.vector.tensor_tensor(
                    lcmr[32:48, :].rearrange("p (c q) -> p c q", q=Q),
                    lv, lv[:, :, 127:128].to_broadcast([16, CPB, Q]),
                    alu.subtract,
                )
                nc.vector.tensor_tensor(
                    lsubr[32:48, :], lcmr[32:48, :], lndt[32:48, :], alu.subtract
                )
                # bounce small per-block vectors through DRAM so they can be
                # partition-broadcast on the way back in
                nc.sync.dma_start(dscr3[0:16, bsl], lcmr[32:48, :])
                nc.sync.dma_start(dscr3[16:32, bsl], lsubr[32:48, :])
                nc.sync.dma_start(dscr2[:, bsl], wc2[32:48, :])
                nc.sync.dma_start(
                    dscr[32:48, b * CPB : (b + 1) * CPB],
                    texp_cm[32:48, :, :].rearrange("p c one -> p (c one)"),
                )
                for k in range(4):
                    nc.sync.dma_start(
                        texp_st[32 * k : 32 * k + 16, :, b * CPB : (b + 1) * CPB],
                        bass.AP(dscr.tensor,
                                dscr.offset + (32 + k) * L + b * CPB,
                                [[0, 16], [4 * L, 4], [1, CPB]]),
                    )

            # ================= phase 2: z + conv (all silus) =================
            tc.tile_set_cur_wait(1.0)
            for b in range(NBLK):
                bsl = slice(b * BLK, (b + 1) * BLK)
                xtb = xtp.tile([128, 8, BLK], f8, tag="xtb")
                nc.sync.dma_start(xtb, xt[:, :, bsl])
                for zt in range(8):
                    ps = [pp_mm, pp_ytp][zt % 2].tile(
                        [128, BLK], f32, tag=["mm", "ytp"][zt % 2])
                    for kj in range(4):
                        nc.tensor.matmul(
                            ps, wt_sb[:, 2 * kj : 2 * kj + 2, 1072 + zt * 128 : 1200 + zt * 128],
                            xtb[:, 2 * kj : 2 * kj + 2, :],
                            start=(kj == 0), stop=(kj == 3), perf_mode=DR,
                        )
                    nc.scalar.activation(zs[:, zt, bsl], ps, AF.Silu, scale=0.0625)
                for ct in range(9):
                    m = 128 if ct < 8 else 32
                    ps = [pp_mm, pp_ytp][ct % 2].tile(
                        [128, BLK], f32, tag=["mm", "ytp"][ct % 2])
                    for k in range(4):
                        a = b * BLK + k
                        mov = (xsr[:, ct, a : a + BLK] if ct < 8
                               else bcr[:, a : a + BLK])
                        nc.tensor.matmul(
                            ps[:m, :], convd[:m, ct * 4 + k, :m], mov,
                            start=(k == 0), stop=(k == 3),
                        )
                    dst = xs_sb[:, ct, bsl] if ct < 8 else bct[:, bsl]
                    nc.scalar.activation(
                        dst, ps[:m, :], AF.Silu, bias=cb_sb[:m, ct : ct + 1]
                    )
                # C rows replicated to the four 32-aligned bases
                for k4 in range(4):
                    nc.sync.dma_start(ct4[32 * k4 : 32 * k4 + 16, bsl], bct[16:32, bsl])
                # C'' = C * exp(l_h) per head
                wc2rep = wcrp.tile([128, 4, BLK], bf16, tag="wc2rep")
                _qw = [nc.scalar, nc.gpsimd, nc.sync, nc.gpsimd]
                for k in range(4):
                    _qw[k].dma_start(
                        wc2rep[32 * k : 32 * k + 16, :, :],
                        bass.AP(dscr2.tensor, dscr2.offset + k * L + b * BLK,
                                [[0, 16], [4 * L, 4], [1, BLK]]),
                    )
                nc.vector.tensor_tensor(
                    c2t[:, :, bsl],
                    ct4[:, bsl].rearrange("p (one c) -> p one c", one=1)
                    .to_broadcast([128, 4, BLK]),
                    wc2rep, alu.mult,
                )

            # ================= phase 3: scan chunks =================
            tc.tile_set_cur_wait(2.0)
            for c in range(NCH):
                cc = c % CPB
                b = c // CPB
                qs = slice(c * Q, (c + 1) * Q)
                if cc == 0:
                    y2 = y2blk.tile([128, 8, BLK], f8, tag="y2")
                # xpos: PE-transpose xs chunk to position-major
                xposr = chkp.tile([128, 8, Q], bf16, tag="xposr")
                for w in range(2):
                    tp = pp_ytp.tile([128, 512], f32, tag="ytp")
                    tpb = tp.bitcast(bf16)
                    for ct in range(4):
                        nc.tensor.transpose(
                            tpb[:, ct * 128 : ct * 128 + 128],
                            xs_sb[:, w * 4 + ct, qs], ident_b,
                        )
                    nc.scalar.copy(xposr[:, w * 4 : w * 4 + 4, :], tpb[:, 0:512])
                xpv = xposr.rearrange("p t c -> p (t c)").rearrange(
                    "p (h c) -> p h c", c=P
                )
                # S psum + B transpose share one bank
                sps = pp_s.tile([128, 512], f32, tag="sps")
                nc.vector.memset(sps[:, 0:256], 0.0)
                # B position-major (bf16 view of spare sps columns)
                bpp = sps.bitcast(bf16)
                nc.tensor.transpose(bpp[:, 576:592], bct[0:16, qs], ident_b[0:16, 0:16])
                bpos = chkp.tile([128, NST], bf16, tag="bpos")
                nc.vector.tensor_copy(bpos, bpp[:, 576:592])
                # head-shared C@B^T -> G0^T[j, i], masked below the diagonal
                nc.tensor.matmul(
                    sps[:, 384:512], bct[0:16, qs], ct4[0:16, qs],
                    start=True, stop=True,
                )
                g0m = chkp.tile([128, Q], bf16, tag="g0m")
                nc.vector.tensor_tensor(g0m, sps[:, 384:512], tril01, alu.mult)
                # gdiff[j,i] = l_i - l_j + ln dt_j via K=2 fp16 matmuls
                # (chunk-recentered rows from dscr3; the shift cancels).
                # exp overflows to +inf above the diagonal; min(.,100) then
                # the g0m mask zeroes those entries.
                lsg = lsgp.tile([2, NH, Q], f16, tag="lsg")  # p0: data, p1: 1
                rsg = lsgp.tile([2, NH, Q], f16, tag="rsg")  # p0: -1, p1: data
                if c < 2:
                    nc.gpsimd.memset(lsg, 1.0)
                    nc.gpsimd.memset(rsg, -1.0)
                nc.gpsimd.dma_start(
                    lsg.rearrange("p h q -> p (h q)")[0:1, :], dscr3[16:32, qs]
                )
                nc.gpsimd.dma_start(
                    rsg.rearrange("p h q -> p (h q)")[1:2, :], dscr3[0:16, qs]
                )
                gdec = gdecp.tile([128, NH, Q], bf16, tag="gdec")
                gsb = gsbp.tile([128, NH, Q], bf16, tag="gsb")
                for g in range(4):
                    gdp = pp_gd.tile([128, 512], f32, tag="gd")
                    for hh_ in range(4):
                        h = g * 4 + hh_
                        lhsT = bass.AP(
                            lsg.tensor, lsg.offset + h * Q,
                            [[NH * Q, 2], [1, Q]],
                        )
                        rhs = bass.AP(
                            rsg.tensor, rsg.offset + h * Q,
                            [[NH * Q, 2], [1, Q]],
                        )
                        nc.tensor.matmul(
                            gdp[:, hh_ * 128 : hh_ * 128 + 128], lhsT, rhs,
                            start=True, stop=True, tile_position=(0, 0),
                        )
                    hsl = slice(g * 4, g * 4 + 4)
                    nc.scalar.activation(gdec[:, hsl, :], gdp, AF.Exp)
                    nc.vector.scalar_tensor_tensor(
                        gsb[:, hsl, :], gdec[:, hsl, :], 100.0,
                        g0m.rearrange("p (one i) -> p one i", one=1)
                        .to_broadcast([128, 4, Q]),
                        alu.min, alu.mult,
                    )
                # wS column = gdec[:, :, last] = dt_j exp(T - l_j); B_ws
                bws = chkp.tile([128, NH, NST], bf16, tag="bws")
                nc.vector.tensor_tensor(
                    bws,
                    bpos.rearrange("p (one n) -> p one n", one=1).to_broadcast([128, NH, NST]),
                    gdec[:, :, 127:128].to_broadcast([128, NH, NST]),
                    alu.mult,
                )
                # Y psums: pairs 0-3 and 4-7
                yA = pp_ytp.tile([128, 512], f32, tag="ytp")
                yB = pp_ytp.tile([128, 512], f32, tag="ytp")
                ypair = [yA, yB]
                for h in range(NH):
                    k, t = h % 4, h // 4
                    # S^T = B_ws.T @ X_h
                    nc.tensor.matmul(
                        sps[32 * k : 32 * k + 16, t * 64 : t * 64 + 64],
                        bws[:, h, :], xpv[:, h, :],
                        start=True, stop=True, tile_position=(0, 32 * k),
                    )
                for h in range(NH):
                    k, t = h % 4, h // 4
                    pr = h // 2
                    # Y^T = X_h.T @ G^T (+ H^T.T @ C''^T)
                    yp = ypair[pr // 4]
                    ysl = (slice(64 * (h % 2), 64 * (h % 2) + 64),
                           slice((pr % 4) * 128, (pr % 4) * 128 + 128))
                    nc.tensor.matmul(
                        yp[ysl[0], ysl[1]], xpv[:, h, :], gsb[:, h, :],
                        start=True, stop=(c == 0),
                    )
                    if c > 0:
                        nc.tensor.matmul(
                            yp[ysl[0], ysl[1]],
                            st[c % 2][32 * k : 32 * k + 16, t, :],
                            c2t[32 * k : 32 * k + 16, t, qs],
                            start=False, stop=True,
                            tile_position=(32 * k, 64 * (h % 2)),
                        )
                # state recurrence
                for t in range(4):
                    nc.vector.scalar_tensor_tensor(
                        st[(c + 1) % 2][:, t, :], st[c % 2][:, t, :],
                        texp_st[:, t, c : c + 1], sps[:, t * 64 : t * 64 + 64],
                        alu.mult, alu.add,
                    )
                # ---------- gating (y8 = gated-y/8 in fp8) ----------
                y8 = y8p.tile([128, 8, Q], f8, tag="y8")
                tmp = gp2.tile([128, 4, Q], f32, tag="gtmp")
                for half in range(2):
                    yp = ypair[half]
                    hsl4 = slice(half * 4, half * 4 + 4)
                    for pr4 in range(4):
                        pr = half * 4 + pr4
                        nc.vector.scalar_tensor_tensor(
                            tmp[:, pr4, :], xs_sb[:, pr, qs],
                            hp_sb[:, 2 + pr : 3 + pr],
                            yp[:, pr4 * 128 : pr4 * 128 + 128],
                            alu.mult, alu.add,
                        )
                    nc.vector.scalar_tensor_tensor(
                        y8[:, hsl4, :], tmp, 0.125, zs[:, hsl4, qs],
                        alu.mult, alu.mult,
                    )
                    nc.gpsimd.tensor_tensor(
                        y2[:, hsl4, cc * Q : cc * Q + Q], y8[:, hsl4, :],
                        y8[:, hsl4, :], alu.mult,
                    )
                # ---------- U matmuls (fp8 DoubleRow) + store ----------
                for oc in range(2):
                    ups = pp_mm.tile([128, BLK], f32, tag="mm")
                    for cj in range(4):
                        nc.tensor.matmul(
                            ups, y8[:, 2 * cj : 2 * cj + 2, :],
                            mft_sb[:, 2 * cj : 2 * cj + 2, oc * 512 : oc * 512 + 512],
                            start=(cj == 0), stop=(cj == 3), perf_mode=DR,
                        )
                    usb = y2p.tile([128, BLK], bf16, tag="usb")
                    nc.scalar.copy(usb, ups)
                    nc.gpsimd.dma_start(
                        u[c * 128 : c * 128 + 128, oc * 512 : oc * 512 + 512], usb
                    )
                # block sumsq
                if cc == CPB - 1:
                    ssps = pp_mm.tile([128, BLK], f32, tag="mm")
                    for ct in range(8):
                        nc.tensor.matmul(
                            ssps[0:1, :], onescol, y2[:, ct, :],
                            start=(ct == 0), stop=(ct == 7),
                        )
                    ssb = y2p.tile([1, BLK], f32, tag="ssb")
                    nc.vector.tensor_copy(ssb, ssps[0:1, :])
                    nc.sync.dma_start(s[0:1, b * BLK : (b + 1) * BLK], ssb)

    nc.finalize()
    return nc


def _get_nc():
    if "nc" not in _cache:
        _cache["nc"] = _build_nc()
    return _cache["nc"]


def _prep_core_inputs(inputs, b, d, hh):
    import ml_dtypes

    fp8 = ml_dtypes.float8_e4m3

    def to8(a):
        return np.clip(a, -240.0, 240.0).astype(fp8)

    pre = "fwd" if d == 0 else "bwd"
    W = np.asarray(inputs[f"{pre}_in_proj_w"], np.float32)  # (4160, 1024)
    x = np.asarray(inputs["x"], np.float32)[b]  # (L, 1024)
    if d == 1:
        x = x[::-1]
    # x^T as (128, 8, L)
    xtv = np.ascontiguousarray(x.T.reshape(8, 128, L).transpose(1, 0, 2))
    # in_proj^T columns: [xs 1024 | B 16 | C 16 | dt 16 | z 1024], x16 for fp8
    W_xs = W[2048 + hh * 1024 : 3072 + hh * 1024]
    W_B = W[4096:4112]
    W_C = W[4112:4128]
    W_dt = W[4128 + hh * 16 : 4144 + hh * 16]
    W_z = W[hh * 1024 : 1024 + hh * 1024]
    Wt = np.concatenate([W_xs, W_B, W_C, W_dt, W_z], axis=0).T * 16.0  # (1024, 2096)
    wtv = np.ascontiguousarray(Wt.reshape(8, 128, 2096).transpose(1, 0, 2))
    # fused output projection, x64 for fp8; y8 carries 1/8 -> U is 8x
    Wo = np.asarray(inputs[f"{pre}_out_proj_w"], np.float32)  # (1024, 2048)
    Wl = np.asarray(inputs["layer_out_proj_w"], np.float32)  # (1024, 2048)
    nw = np.asarray(inputs[f"{pre}_norm_w"], np.float32)
    ch = slice(hh * 1024, hh * 1024 + 1024)
    M = (Wl[:, d * 1024 : d * 1024 + 1024] @ Wo)[:, ch] * nw[ch][None, :]
    MfT = M.T * 64.0  # (1024 c, 1024 o)
    mftv = np.ascontiguousarray(MfT.reshape(8, 128, 1024).transpose(1, 0, 2))
    # conv: diag values per (c-tile, tap) column
    cwf = np.asarray(inputs[f"{pre}_conv_w"], np.float32)[:, 0, :]  # (2080, 4)
    cwl = np.concatenate([cwf[hh * 1024 : 1024 + hh * 1024], cwf[2048:2080]], axis=0)
    cwv = np.zeros((128, 36), np.float32)
    for ct in range(9):
        n = 128 if ct < 8 else 32
        for k in range(4):
            cwv[:n, ct * 4 + k] = cwl[ct * 128 : ct * 128 + n, k]
    cbf = np.asarray(inputs[f"{pre}_conv_b"], np.float32)
    cbl = np.concatenate([cbf[hh * 1024 : 1024 + hh * 1024], cbf[2048:2080]])
    cbv = np.zeros((128, 18), np.float32)
    for ct in range(9):
        n = 128 if ct < 8 else 32
        cbv[:n, ct] = cbl[ct * 128 : ct * 128 + n]
    # host params
    hpv = np.zeros((128, 18), np.float32)
    hs = slice(hh * 16, hh * 16 + 16)
    hpv[32:48, 0] = np.asarray(inputs[f"{pre}_dt_bias"], np.float32)[hs]
    hpv[32:48, 1] = -np.exp(np.asarray(inputs[f"{pre}_A_log"], np.float32)[hs])
    Dp = np.asarray(inputs[f"{pre}_Dp"], np.float32)[hs]
    for h in range(NH):
        hpv[:, 2 + h] = Dp[h]
    return {
        "xt": to8(xtv),
        "wt": to8(wtv),
        "mft": to8(mftv),
        "cw": cwv,
        "cb": cbv,
        "hp": hpv,
    }


def _combine(inputs, results):
    x = np.asarray(inputs["x"], np.float32)
    scale = np.asarray(inputs["layer_scale"], np.float32)
    out = x.copy()
    i = 0
    for b in range(2):
        for d in range(2):
            U0 = np.asarray(results[i]["u"], np.float32)
            s0 = results[i]["s"][0]
            U1 = np.asarray(results[i + 1]["u"], np.float32)
            s1 = results[i + 1]["s"][0]
            i += 2
            r = 1.0 / np.sqrt((s0 + s1) / 32.0 + EPS)
            contrib = r[:, None] * (U0 + U1) / 8.0
            if d == 1:
                contrib = contrib[::-1]
            out[b] += contrib * scale[None, :]
    return out


def _run(inputs, trace=False, core_ids=None):
    from concourse.bass_utils import run_bass_kernel_spmd

    nc = _get_nc()
    in_maps = []
    for b in range(2):
        for d in range(2):
            for hh in range(2):
                in_maps.append(_prep_core_inputs(inputs, b, d, hh))
    if core_ids is None:
        core_ids = list(range(8))
    res = run_bass_kernel_spmd(
        nc, in_maps[: len(core_ids)], core_ids=core_ids, trace=trace
    )
    return res


def kernel(**inputs):
    res = _run(inputs)
    return _combine(inputs, res.results)


# revision 50
# speedup vs baseline: 1.1848x; 1.0038x over previous
"""BiMambaLayer Trainium2 kernel.

Sharding: 8 cores = batch(2) x direction(2) x head-half(2). Each core runs the
full L=2048 sequence of one (batch, direction) through 16 of the 32 heads of
that direction's Mamba2 block, plus the fused output projection restricted to
its 1024 d_inner channels. The gated-RMSNorm row scale commutes with the output
projections, so each core returns an unnormalized partial projection U and a
partial sum-of-squares s; the host combines:
    out[b] = x[b] + scale * sum_dir flip_d( r_d[:,None] * (U0 + U1) / 8 ),
    r_d = rsqrt((s0 + s1)/32 + eps).

Three phases over the whole sequence (so the activation table never thrashes
between the exp/ln set and the silu set, and the PE stays warm through dense
GEMM bursts):
  1) in_proj (fp8 DoubleRow) + dt pipeline for all 4 blocks of 512,
  2) z-proj (fp8 DoubleRow) + causal conv (fp8 diag matmuls) + all silus,
  3) 16 scan chunks of 128: head-shared C@B^T, per-head decay
     exp(l_i - l_j + ln dt_j) with the tri mask applied as min(exp,100)*g0m
     (masked entries overflow exp to +inf; min() tames them before the 0
     mask), Y^T = X^T.T@G^T + D_h*X^T + H^T.T@C''^T in PSUM, chunk-state
     recurrence on DVE, gating y8 = y*silu(z)/8 in fp8, U = y8 @ Mfused
     (fp8 DoubleRow).
fp8 scaling: in_proj weights x16 (undone at PSUM read), Mfused x64, y x1/8;
the host combine divides U by 8 and rescales s accordingly.
"""
import numpy as np

L = 2048
DM = 1024  # d_model
Q = 128  # scan chunk
NCH = L // Q  # 16 chunks
BLK = 512
NBLK = L // BLK  # 4
CPB = BLK // Q  # 4 chunks per block
NH = 16  # local heads
P = 64  # head dim
NST = 16  # state dim
EPS = 1e-5

_cache = {}


def _build_nc():
    import concourse.bass as bass
    import concourse.tile as tile
    import concourse.mybir as mybir
    from concourse import bacc
    from concourse.masks import make_identity
    from concourse.alu_op_type import AluOpType as alu

    f32 = mybir.dt.float32
    f16 = mybir.dt.float16
    bf16 = mybir.dt.bfloat16
    f8 = mybir.dt.float8e4
    AF = mybir.ActivationFunctionType
    DR = mybir.MatmulPerfMode.DoubleRow

    nc = bacc.Bacc(trn_type="TRN2")

    # ---- DRAM I/O (per-core shapes; host prepares layouts) ----
    xt = nc.dram_tensor("xt", [128, 8, L], f8, kind="ExternalInput")
    wt = nc.dram_tensor("wt", [128, 8, 2096], f8, kind="ExternalInput")
    mft = nc.dram_tensor("mft", [128, 8, DM], f8, kind="ExternalInput")
    cw = nc.dram_tensor("cw", [128, 36], f32, kind="ExternalInput")
    cb = nc.dram_tensor("cb", [128, 18], f32, kind="ExternalInput")
    hp = nc.dram_tensor("hp", [128, 18], f32, kind="ExternalInput")
    u = nc.dram_tensor("u", [L, DM], bf16, kind="ExternalOutput")
    s = nc.dram_tensor("s", [1, L], f32, kind="ExternalOutput")

    from contextlib import ExitStack

    with tile.TileContext(nc) as tc, ExitStack() as ctx:
        ep = ctx.enter_context
        const = ep(tc.tile_pool(name="const", bufs=1))
        seqp = ep(tc.tile_pool(name="seqp", bufs=1))
        statep = ep(tc.tile_pool(name="state", bufs=1))
        xtp = ep(tc.tile_pool(name="xtp", bufs=2))
        dtlp = ep(tc.tile_pool(name="dtlp", bufs=2))
        wcrp = ep(tc.tile_pool(name="wcrp", bufs=1))
        dscrp = ep(tc.tile_pool(name="dscrp", bufs=1, space="DRAM"))
        gp2 = ep(tc.tile_pool(name="gp2", bufs=2))
        lsgp = ep(tc.tile_pool(name="lsgp", bufs=2))
        chkp = ep(tc.tile_pool(name="chkp", bufs=3))
        gdecp = ep(tc.tile_pool(name="gdecp", bufs=2))
        gsbp = ep(tc.tile_pool(name="gsbp", bufs=2))
        y2p = ep(tc.tile_pool(name="y2p", bufs=2))
        y8p = ep(tc.tile_pool(name="y8p", bufs=2))
        y2blk = ep(tc.tile_pool(name="y2blk", bufs=1))
        pp_mm = ep(tc.tile_pool(name="pp_mm", bufs=2, space="PSUM"))
        pp_ytp = ep(tc.tile_pool(name="pp_ytp", bufs=3, space="PSUM"))
        pp_s = ep(tc.tile_pool(name="pp_s", bufs=2, space="PSUM"))
        pp_gd = ep(tc.tile_pool(name="pp_gd", bufs=1, space="PSUM"))
        if True:
            # ---------- constants / persistent ----------
            wt_sb = const.tile([128, 8, 2096], f8)
            nc.sync.dma_start(wt_sb, wt[:, :, :])
            mft_sb = const.tile([128, 8, DM], f8)
            nc.sync.dma_start(mft_sb, mft[:, :, :])
            cw_sb = const.tile([128, 36], f32)
            nc.sync.dma_start(cw_sb, cw[:, :])
            cb_sb = const.tile([128, 18], f32)
            nc.sync.dma_start(cb_sb, cb[:, :])
            hp_sb = const.tile([128, 18], f32)
            nc.sync.dma_start(hp_sb, hp[:, :])

            ident_b = const.tile([128, 128], bf16)
            make_identity(nc, ident_b)
            ident_f = const.tile([128, 128], f32)
            make_identity(nc, ident_f)
            # multiplicative mask, [j, i] coords: 1 where i >= j, 0 where i < j
            tril01 = const.tile([128, 128], bf16)
            nc.gpsimd.memset(tril01, 1.0)
            nc.gpsimd.affine_select(
                out=tril01, in_=tril01, compare_op=mybir.AluOpType.is_ge,
                fill=0.0, base=0, pattern=[[1, 128]], channel_multiplier=-1,
            )
            onesq = const.tile([128, 128], f32)
            nc.vector.memset(onesq, 1.0)
            onescol = const.tile([128, 1], f8)
            nc.vector.memset(onescol, 1.0)
            # conv diagonal weight tiles (fp8), built from cw columns
            convd = const.tile([128, 36, 128], f8)
            for j in range(36):
                nc.scalar.mul(convd[:, j, :], ident_b, cw_sb[:, j : j + 1])
            # persistent scan state: 4 head-groups (heads 4t+k at partitions
            # 32k..32k+16), ping-pong A/B
            stA = statep.tile([128, 4, P], f8, tag="stA")
            stB = statep.tile([128, 4, P], f8, tag="stB")
            nc.vector.memset(stA, 0.0)
            nc.vector.memset(stB, 0.0)
            st = [stA, stB]
            # chunk-decay per-partition scalars, [state-tile, chunk]
            texp_st = statep.tile([128, 4, NCH], f32, tag="texp")
            nc.vector.memset(texp_st, 0.0)

            # whole-sequence activations
            xsr = seqp.tile([128, 8, L + 3], f8)  # in_proj xs (+3 zero halo)
            bcr = seqp.tile([32, L + 3], f8)
            xs_sb = seqp.tile([128, 8, L], bf16)  # conv+silu out
            bct = seqp.tile([32, L], bf16)
            ct4 = seqp.tile([128, L], bf16)  # C rows replicated to 4 bases
            zs = seqp.tile([128, 8, L], f8)  # silu(z)
            c2t = seqp.tile([128, 4, L], f8)  # C * exp(l_h) per head
            nc.vector.memset(xsr[:, :, 0:3], 0.0)
            nc.vector.memset(bcr[:, 0:3], 0.0)

            dscr = dscrp.tile([48, L], f32, tag="dscr")
            dscr2 = dscrp.tile([16, L], bf16, tag="dscr2")
            dscr3 = dscrp.tile([32, L], f16, tag="dscr3")

            # ================= phase 1: in_proj + dt =================
            for b in range(NBLK):
                bsl = slice(b * BLK, (b + 1) * BLK)
                bsl3 = slice(3 + b * BLK, 3 + (b + 1) * BLK)
                xtb = xtp.tile([128, 8, BLK], f8, tag="xtb")
                nc.sync.dma_start(xtb, xt[:, :, bsl])
                dt_sp = dtlp.tile([128, BLK], f32, tag="dtsp")
                for et in range(9):
                    m = 128 if et < 8 else 48
                    ecol = et * 128 if et < 8 else 1024
                    ps = [pp_mm, pp_ytp][et % 2].tile(
                        [128, BLK], f32, tag=["mm", "ytp"][et % 2])
                    for kj in range(4):
                        nc.tensor.matmul(
                            ps[:m, :], wt_sb[:, 2 * kj : 2 * kj + 2, ecol : ecol + m],
                            xtb[:, 2 * kj : 2 * kj + 2, :],
                            start=(kj == 0), stop=(kj == 3), perf_mode=DR,
                        )
                    if et < 8:
                        if et % 2 == 0:
                            nc.scalar.mul(xsr[:, et, bsl3], ps, 0.0625)
                        else:
                            nc.vector.tensor_scalar_mul(
                                xsr[:, et, bsl3], ps, 0.0625)
                    else:
                        nc.scalar.mul(bcr[:, bsl3], ps[0:32, :], 0.0625)
                        nc.scalar.activation(
                            dt_sp[32:48, :], ps[32:48, :], AF.Exp,
                            bias=hp_sb[32:48, 0:1], scale=0.0625,
                        )
                        nc.vector.tensor_scalar_add(
                            dt_sp[32:48, :], dt_sp[32:48, :], 1.0
                        )
                        nc.scalar.activation(dt_sp[32:48, :], dt_sp[32:48, :], AF.Ln)
                # ---------- dt pipeline ----------
                lndt = dtlp.tile([128, BLK], f32, tag="lndt")
                lcm = dtlp.tile([128, BLK], f32, tag="lcm")
                wc2 = dtlp.tile([128, BLK], bf16, tag="wc2")
                nc.scalar.activation(lndt[32:48, :], dt_sp[32:48, :], AF.Ln)
                dtA = dtlp.tile([128, BLK], f32, tag="dtA")
                nc.vector.tensor_scalar_mul(
                    dtA[32:48, :], dt_sp[32:48, :], hp_sb[32:48, 1:2]
                )
                for cc in range(CPB):
                    qs = slice(cc * Q, (cc + 1) * Q)
                    nc.vector.tensor_tensor_scan(
                        lcm[32:48, qs], onesq[32:48, :], dtA[32:48, qs],
                        0.0, alu.mult, alu.add,
                    )
                nc.scalar.activation(wc2[32:48, :], lcm[32:48, :], AF.Exp)
                texp_cm = dtlp.tile([128, CPB, 1], f32, tag="texpcm")
                lv = lcm[32:48, :].rearrange("p (c q) -> p c q", q=Q)
                nc.scalar.activation(texp_cm[32:48, :, :], lv[:, :, 127:128], AF.Exp)
                # chunk-recentered l rows in fp16 for the gdiff matmuls
                lcmr = dtlp.tile([128, BLK], f16, tag="lcmr")
                lsubr = dtlp.tile([128, BLK], f16, tag="lsubr")
                nc.vector.tensor_tensor(
                    lcmr[32:48, :].rearrange("p (c q) -> p c q", q=Q),
                    lv, lv[:, :, 127:128].to_broadcast([16, CPB, Q]),
                    alu.subtract,
                )
                nc.vector.tensor_tensor(
                    lsubr[32:48, :], lcmr[32:48, :], lndt[32:48, :], alu.subtract
                )
                # bounce small per-block vectors through DRAM so they can be
                # partition-broadcast on the way back in
                nc.sync.dma_start(dscr3[0:16, bsl], lcmr[32:48, :])
                nc.sync.dma_start(dscr3[16:32, bsl], lsubr[32:48, :])
                nc.sync.dma_start(dscr2[:, bsl], wc2[32:48, :])
                nc.sync.dma_start(
                    dscr[32:48, b * CPB : (b + 1) * CPB],
                    texp_cm[32:48, :, :].rearrange("p c one -> p (c one)"),
                )
                for k in range(4):
                    nc.sync.dma_start(
                        texp_st[32 * k : 32 * k + 16, :, b * CPB : (b + 1) * CPB],
                        bass.AP(dscr.tensor,
                                dscr.offset + (32 + k) * L + b * CPB,
                                [[0, 16], [4 * L, 4], [1, CPB]]),
                    )

            # ================= phase 2: z + conv (all silus) =================
            tc.tile_set_cur_wait(1.0)
            for b in range(NBLK):
                bsl = slice(b * BLK, (b + 1) * BLK)
                xtb = xtp.tile([128, 8, BLK], f8, tag="xtb")
                nc.sync.dma_start(xtb, xt[:, :, bsl])
                for zt in range(8):
                    ps = [pp_mm, pp_ytp][zt % 2].tile(
                        [128, BLK], f32, tag=["mm", "ytp"][zt % 2])
                    for kj in range(4):
                        nc.tensor.matmul(
                            ps, wt_sb[:, 2 * kj : 2 * kj + 2, 1072 + zt * 128 : 1200 + zt * 128],
                            xtb[:, 2 * kj : 2 * kj + 2, :],
                            start=(kj == 0), stop=(kj == 3), perf_mode=DR,
                        )
                    nc.scalar.activation(zs[:, zt, bsl], ps, AF.Silu, scale=0.0625)
                for ct in range(9):
                    m = 128 if ct < 8 else 32
                    ps = [pp_mm, pp_ytp][ct % 2].tile(
                        [128, BLK], f32, tag=["mm", "ytp"][ct % 2])
                    for k in range(4):
                        a = b * BLK + k
                        mov = (xsr[:, ct, a : a + BLK] if ct < 8
                               else bcr[:, a : a + BLK])
                        nc.tensor.matmul(
                            ps[:m, :], convd[:m, ct * 4 + k, :m], mov,
                            start=(k == 0), stop=(k == 3),
                        )
                    dst = xs_sb[:, ct, bsl] if ct < 8 else bct[:, bsl]
                    nc.scalar.activation(
                        dst, ps[:m, :], AF.Silu, bias=cb_sb[:m, ct : ct + 1]
                    )
                # C rows replicated to the four 32-aligned bases
                for k4 in range(4):
                    nc.sync.dma_start(ct4[32 * k4 : 32 * k4 + 16, bsl], bct[16:32, bsl])
                # C'' = C * exp(l_h) per head
                wc2rep = wcrp.tile([128, 4, BLK], bf16, tag="wc2rep")
                _qw = [nc.scalar, nc.gpsimd, nc.sync, nc.gpsimd]
                for k in range(4):
                    _qw[k].dma_start(
                        wc2rep[32 * k : 32 * k + 16, :, :],
                        bass.AP(dscr2.tensor, dscr2.offset + k * L + b * BLK,
                                [[0, 16], [4 * L, 4], [1, BLK]]),
                    )
                nc.vector.tensor_tensor(
                    c2t[:, :, bsl],
                    ct4[:, bsl].rearrange("p (one c) -> p one c", one=1)
                    .to_broadcast([128, 4, BLK]),
                    wc2rep, alu.mult,
                )

            # ================= phase 3: scan chunks =================
            for c in range(NCH):
                cc = c % CPB
                b = c // CPB
                qs = slice(c * Q, (c + 1) * Q)
                if cc == 0:
                    y2 = y2blk.tile([128, 8, BLK], f8, tag="y2")
                # xpos: PE-transpose xs chunk to position-major
                xposr = chkp.tile([128, 8, Q], bf16, tag="xposr")
                for w in range(2):
                    tp = pp_ytp.tile([128, 512], f32, tag="ytp")
                    tpb = tp.bitcast(bf16)
                    for ct in range(4):
                        nc.tensor.transpose(
                            tpb[:, ct * 128 : ct * 128 + 128],
                            xs_sb[:, w * 4 + ct, qs], ident_b,
                        )
                    nc.scalar.copy(xposr[:, w * 4 : w * 4 + 4, :], tpb[:, 0:512])
                xpv = xposr.rearrange("p t c -> p (t c)").rearrange(
                    "p (h c) -> p h c", c=P
                )
                # S psum + B transpose share one bank
                sps = pp_s.tile([128, 512], f32, tag="sps")
                nc.vector.memset(sps[:, 0:256], 0.0)
                # B position-major (bf16 view of spare sps columns)
                bpp = sps.bitcast(bf16)
                nc.tensor.transpose(bpp[:, 576:592], bct[0:16, qs], ident_b[0:16, 0:16])
                bpos = chkp.tile([128, NST], bf16, tag="bpos")
                nc.vector.tensor_copy(bpos, bpp[:, 576:592])
                # head-shared C@B^T -> G0^T[j, i], masked below the diagonal
                nc.tensor.matmul(
                    sps[:, 384:512], bct[0:16, qs], ct4[0:16, qs],
                    start=True, stop=True,
                )
                g0m = chkp.tile([128, Q], bf16, tag="g0m")
                nc.vector.tensor_tensor(g0m, sps[:, 384:512], tril01, alu.mult)
                # gdiff[j,i] = l_i - l_j + ln dt_j via K=2 fp16 matmuls
                # (chunk-recentered rows from dscr3; the shift cancels).
                # exp overflows to +inf above the diagonal; min(.,100) then
                # the g0m mask zeroes those entries.
                lsg = lsgp.tile([2, NH, Q], f16, tag="lsg")  # p0: data, p1: 1
                rsg = lsgp.tile([2, NH, Q], f16, tag="rsg")  # p0: -1, p1: data
                if c < 2:
                    nc.gpsimd.memset(lsg, 1.0)
                    nc.gpsimd.memset(rsg, -1.0)
                nc.gpsimd.dma_start(
                    lsg.rearrange("p h q -> p (h q)")[0:1, :], dscr3[16:32, qs]
                )
                nc.gpsimd.dma_start(
                    rsg.rearrange("p h q -> p (h q)")[1:2, :], dscr3[0:16, qs]
                )
                gdec = gdecp.tile([128, NH, Q], bf16, tag="gdec")
                gsb = gsbp.tile([128, NH, Q], bf16, tag="gsb")
                for g in range(4):
                    gdp = pp_gd.tile([128, 512], f32, tag="gd")
                    for hh_ in range(4):
                        h = g * 4 + hh_
                        lhsT = bass.AP(
                            lsg.tensor, lsg.offset + h * Q,
                            [[NH * Q, 2], [1, Q]],
                        )
                        rhs = bass.AP(
                            rsg.tensor, rsg.offset + h * Q,
                            [[NH * Q, 2], [1, Q]],
                        )
                        nc.tensor.matmul(
                            gdp[:, hh_ * 128 : hh_ * 128 + 128], lhsT, rhs,
                            start=True, stop=True, tile_position=(0, 0),
                        )
                    hsl = slice(g * 4, g * 4 + 4)
                    nc.scalar.activation(gdec[:, hsl, :], gdp, AF.Exp)
                    nc.vector.scalar_tensor_tensor(
                        gsb[:, hsl, :], gdec[:, hsl, :], 100.0,
                        g0m.rearrange("p (one i) -> p one i", one=1)
                        .to_broadcast([128, 4, Q]),
                        alu.min, alu.mult,
                    )
                # wS column = gdec[:, :, last] = dt_j exp(T - l_j); B_ws
                bws = chkp.tile([128, NH, NST], bf16, tag="bws")
                nc.vector.tensor_tensor(
                    bws,
                    bpos.rearrange("p (one n) -> p one n", one=1).to_broadcast([128, NH, NST]),
                    gdec[:, :, 127:128].to_broadcast([128, NH, NST]),
                    alu.mult,
                )
                # Y psums: pairs 0-3 and 4-7
                yA = pp_ytp.tile([128, 512], f32, tag="ytp")
                yB = pp_ytp.tile([128, 512], f32, tag="ytp")
                ypair = [yA, yB]
                for h in range(NH):
                    k, t = h % 4, h // 4
                    # S^T = B_ws.T @ X_h
                    nc.tensor.matmul(
                        sps[32 * k : 32 * k + 16, t * 64 : t * 64 + 64],
                        bws[:, h, :], xpv[:, h, :],
                        start=True, stop=True, tile_position=(0, 32 * k),
                    )
                for h in range(NH):
                    k, t = h % 4, h // 4
                    pr = h // 2
                    # Y^T = X_h.T @ G^T (+ H^T.T @ C''^T)
                    yp = ypair[pr // 4]
                    ysl = (slice(64 * (h % 2), 64 * (h % 2) + 64),
                           slice((pr % 4) * 128, (pr % 4) * 128 + 128))
                    nc.tensor.matmul(
                        yp[ysl[0], ysl[1]], xpv[:, h, :], gsb[:, h, :],
                        start=True, stop=(c == 0),
                    )
                    if c > 0:
                        nc.tensor.matmul(
                            yp[ysl[0], ysl[1]],
                            st[c % 2][32 * k : 32 * k + 16, t, :],
                            c2t[32 * k : 32 * k + 16, t, qs],
                            start=False, stop=True,
                            tile_position=(32 * k, 64 * (h % 2)),
                        )
                # state recurrence
                for t in range(4):
                    nc.vector.scalar_tensor_tensor(
                        st[(c + 1) % 2][:, t, :], st[c % 2][:, t, :],
                        texp_st[:, t, c : c + 1], sps[:, t * 64 : t * 64 + 64],
                        alu.mult, alu.add,
                    )
                # ---------- gating (y8 = gated-y/8 in fp8) ----------
                y8 = y8p.tile([128, 8, Q], f8, tag="y8")
                tmp = gp2.tile([128, 4, Q], f32, tag="gtmp")
                for half in range(2):
                    yp = ypair[half]
                    hsl4 = slice(half * 4, half * 4 + 4)
                    for pr4 in range(4):
                        pr = half * 4 + pr4
                        nc.vector.scalar_tensor_tensor(
                            tmp[:, pr4, :], xs_sb[:, pr, qs],
                            hp_sb[:, 2 + pr : 3 + pr],
                            yp[:, pr4 * 128 : pr4 * 128 + 128],
                            alu.mult, alu.add,
                        )
                    nc.vector.scalar_tensor_tensor(
                        y8[:, hsl4, :], tmp, 0.125, zs[:, hsl4, qs],
                        alu.mult, alu.mult,
                    )
                    nc.gpsimd.tensor_tensor(
                        y2[:, hsl4, cc * Q : cc * Q + Q], y8[:, hsl4, :],
                        y8[:, hsl4, :], alu.mult,
                    )
                # ---------- U matmuls (fp8 DoubleRow) + store ----------
                for oc in range(2):
                    ups = pp_mm.tile([128, BLK], f32, tag="mm")
                    for cj in range(4):
                        nc.tensor.matmul(
                            ups, y8[:, 2 * cj : 2 * cj + 2, :],
                            mft_sb[:, 2 * cj : 2 * cj + 2, oc * 512 : oc * 512 + 512],
                            start=(cj == 0), stop=(cj == 3), perf_mode=DR,
                        )
                    usb = y2p.tile([128, BLK], bf16, tag="usb")
                    nc.scalar.copy(usb, ups)
                    nc.gpsimd.dma_start(
                        u[c * 128 : c * 128 + 128, oc * 512 : oc * 512 + 512], usb
                    )
                # block sumsq
                if cc == CPB - 1:
                    ssps = pp_mm.tile([128, BLK], f32, tag="mm")
                    for ct in range(8):
                        nc.tensor.matmul(
                            ssps[0:1, :], onescol, y2[:, ct, :],
                            start=(ct == 0), stop=(ct == 7),
                        )
                    ssb = y2p.tile([1, BLK], f32, tag="ssb")
                    nc.vector.tensor_copy(ssb, ssps[0:1, :])
                    nc.sync.dma_start(s[0:1, b * BLK : (b + 1) * BLK], ssb)

    nc.finalize()
    return nc


def _get_nc():
    if "nc" not in _cache:
        _cache["nc"] = _build_nc()
    return _cache["nc"]


def _prep_core_inputs(inputs, b, d, hh):
    import ml_dtypes

    fp8 = ml_dtypes.float8_e4m3

    def to8(a):
        return np.clip(a, -240.0, 240.0).astype(fp8)

    pre = "fwd" if d == 0 else "bwd"
    W = np.asarray(inputs[f"{pre}_in_proj_w"], np.float32)  # (4160, 1024)
    x = np.asarray(inputs["x"], np.float32)[b]  # (L, 1024)
    if d == 1:
        x = x[::-1]
    # x^T as (128, 8, L)
    xtv = np.ascontiguousarray(x.T.reshape(8, 128, L).transpose(1, 0, 2))
    # in_proj^T columns: [xs 1024 | B 16 | C 16 | dt 16 | z 1024], x16 for fp8
    W_xs = W[2048 + hh * 1024 : 3072 + hh * 1024]
    W_B = W[4096:4112]
    W_C = W[4112:4128]
    W_dt = W[4128 + hh * 16 : 4144 + hh * 16]
    W_z = W[hh * 1024 : 1024 + hh * 1024]
    Wt = np.concatenate([W_xs, W_B, W_C, W_dt, W_z], axis=0).T * 16.0  # (1024, 2096)
    wtv = np.ascontiguousarray(Wt.reshape(8, 128, 2096).transpose(1, 0, 2))
    # fused output projection, x64 for fp8; y8 carries 1/8 -> U is 8x
    Wo = np.asarray(inputs[f"{pre}_out_proj_w"], np.float32)  # (1024, 2048)
    Wl = np.asarray(inputs["layer_out_proj_w"], np.float32)  # (1024, 2048)
    nw = np.asarray(inputs[f"{pre}_norm_w"], np.float32)
    ch = slice(hh * 1024, hh * 1024 + 1024)
    M = (Wl[:, d * 1024 : d * 1024 + 1024] @ Wo)[:, ch] * nw[ch][None, :]
    MfT = M.T * 64.0  # (1024 c, 1024 o)
    mftv = np.ascontiguousarray(MfT.reshape(8, 128, 1024).transpose(1, 0, 2))
    # conv: diag values per (c-tile, tap) column
    cwf = np.asarray(inputs[f"{pre}_conv_w"], np.float32)[:, 0, :]  # (2080, 4)
    cwl = np.concatenate([cwf[hh * 1024 : 1024 + hh * 1024], cwf[2048:2080]], axis=0)
    cwv = np.zeros((128, 36), np.float32)
    for ct in range(9):
        n = 128 if ct < 8 else 32
        for k in range(4):
            cwv[:n, ct * 4 + k] = cwl[ct * 128 : ct * 128 + n, k]
    cbf = np.asarray(inputs[f"{pre}_conv_b"], np.float32)
    cbl = np.concatenate([cbf[hh * 1024 : 1024 + hh * 1024], cbf[2048:2080]])
    cbv = np.zeros((128, 18), np.float32)
    for ct in range(9):
        n = 128 if ct < 8 else 32
        cbv[:n, ct] = cbl[ct * 128 : ct * 128 + n]
    # host params
    hpv = np.zeros((128, 18), np.float32)
    hs = slice(hh * 16, hh * 16 + 16)
    hpv[32:48, 0] = np.asarray(inputs[f"{pre}_dt_bias"], np.float32)[hs]
    hpv[32:48, 1] = -np.exp(np.asarray(inputs[f"{pre}_A_log"], np.float32)[hs])
    Dp = np.asarray(inputs[f"{pre}_Dp"], np.float32)[hs]
    for h in range(NH):
        hpv[:, 2 + h] = Dp[h]
    return {
        "xt": to8(xtv),
        "wt": to8(wtv),
        "mft": to8(mftv),
        "cw": cwv,
        "cb": cbv,
        "hp": hpv,
    }


def _combine(inputs, results):
    x = np.asarray(inputs["x"], np.float32)
    scale = np.asarray(inputs["layer_scale"], np.float32)
    out = x.copy()
    i = 0
    for b in range(2):
        for d in range(2):
            U0 = np.asarray(results[i]["u"], np.float32)
            s0 = results[i]["s"][0]
            U1 = np.asarray(results[i + 1]["u"], np.float32)
            s1 = results[i + 1]["s"][0]
            i += 2
            r = 1.0 / np.sqrt((s0 + s1) / 32.0 + EPS)
            contrib = r[:, None] * (U0 + U1) / 8.0
            if d == 1:
                contrib = contrib[::-1]
            out[b] += contrib * scale[None, :]
    return out


def _run(inputs, trace=False, core_ids=None):
    from concourse.bass_utils import run_bass_kernel_spmd

    nc = _get_nc()
    in_maps = []
    for b in range(2):
        for d in range(2):
            for hh in range(2):
                in_maps.append(_prep_core_inputs(inputs, b, d, hh))
    if core_ids is None:
        core_ids = list(range(8))
    res = run_bass_kernel_spmd(
        nc, in_maps[: len(core_ids)], core_ids=core_ids, trace=trace
    )
    return res


def kernel(**inputs):
    res = _run(inputs)
    return _combine(inputs, res.results)


# revision 51
# speedup vs baseline: 1.2156x; 1.0261x over previous
"""BiMambaLayer Trainium2 kernel.

Sharding: 8 cores = batch(2) x direction(2) x head-half(2). Each core runs the
full L=2048 sequence of one (batch, direction) through 16 of the 32 heads of
that direction's Mamba2 block, plus the fused output projection restricted to
its 1024 d_inner channels. The gated-RMSNorm row scale commutes with the output
projections, so each core returns an unnormalized partial projection U and a
partial sum-of-squares s; the host combines:
    out[b] = x[b] + scale * sum_dir flip_d( r_d[:,None] * (U0 + U1) / 8 ),
    r_d = rsqrt((s0 + s1)/32 + eps).

Three phases over the whole sequence (so the activation table never thrashes
between the exp/ln set and the silu set, and the PE stays warm through dense
GEMM bursts):
  1) in_proj (fp8 DoubleRow) + dt pipeline for all 4 blocks of 512,
  2) z-proj (fp8 DoubleRow) + causal conv (fp8 diag matmuls) + all silus,
  3) 16 scan chunks of 128: head-shared C@B^T, per-head decay
     exp(l_i - l_j + ln dt_j) with the tri mask applied as min(exp,100)*g0m
     (masked entries overflow exp to +inf; min() tames them before the 0
     mask), Y^T = X^T.T@G^T + D_h*X^T + H^T.T@C''^T in PSUM, chunk-state
     recurrence on DVE, gating y8 = y*silu(z)/8 in fp8, U = y8 @ Mfused
     (fp8 DoubleRow).
fp8 scaling: in_proj weights x16 (undone at PSUM read), Mfused x64, y x1/8;
the host combine divides U by 8 and rescales s accordingly.
"""
import numpy as np

L = 2048
DM = 1024  # d_model
Q = 128  # scan chunk
NCH = L // Q  # 16 chunks
BLK = 512
NBLK = L // BLK  # 4
CPB = BLK // Q  # 4 chunks per block
NH = 16  # local heads
P = 64  # head dim
NST = 16  # state dim
EPS = 1e-5

_cache = {}


def _build_nc():
    import concourse.bass as bass
    import concourse.tile as tile
    import concourse.mybir as mybir
    from concourse import bacc
    from concourse.masks import make_identity
    from concourse.alu_op_type import AluOpType as alu

    f32 = mybir.dt.float32
    f16 = mybir.dt.float16
    bf16 = mybir.dt.bfloat16
    f8 = mybir.dt.float8e4
    AF = mybir.ActivationFunctionType
    DR = mybir.MatmulPerfMode.DoubleRow

    nc = bacc.Bacc(trn_type="TRN2")

    # ---- DRAM I/O (per-core shapes; host prepares layouts) ----
    xt = nc.dram_tensor("xt", [128, 8, L], f8, kind="ExternalInput")
    wt = nc.dram_tensor("wt", [128, 8, 2096], f8, kind="ExternalInput")
    mft = nc.dram_tensor("mft", [128, 8, DM], f8, kind="ExternalInput")
    cw = nc.dram_tensor("cw", [128, 36], f32, kind="ExternalInput")
    cb = nc.dram_tensor("cb", [128, 18], f32, kind="ExternalInput")
    hp = nc.dram_tensor("hp", [128, 18], f32, kind="ExternalInput")
    u = nc.dram_tensor("u", [L, DM], bf16, kind="ExternalOutput")
    s = nc.dram_tensor("s", [1, L], f32, kind="ExternalOutput")

    from contextlib import ExitStack

    with tile.TileContext(nc) as tc, ExitStack() as ctx:
        ep = ctx.enter_context
        const = ep(tc.tile_pool(name="const", bufs=1))
        seqp = ep(tc.tile_pool(name="seqp", bufs=1))
        statep = ep(tc.tile_pool(name="state", bufs=1))
        xtp = ep(tc.tile_pool(name="xtp", bufs=2))
        dtlp = ep(tc.tile_pool(name="dtlp", bufs=2))
        wcrp = ep(tc.tile_pool(name="wcrp", bufs=1))
        dscrp = ep(tc.tile_pool(name="dscrp", bufs=1, space="DRAM"))
        gp2 = ep(tc.tile_pool(name="gp2", bufs=2))
        lsgp = ep(tc.tile_pool(name="lsgp", bufs=2))
        chkp = ep(tc.tile_pool(name="chkp", bufs=3))
        gdecp = ep(tc.tile_pool(name="gdecp", bufs=2))
        gsbp = ep(tc.tile_pool(name="gsbp", bufs=2))
        y2p = ep(tc.tile_pool(name="y2p", bufs=2))
        y8p = ep(tc.tile_pool(name="y8p", bufs=2))
        y2blk = ep(tc.tile_pool(name="y2blk", bufs=1))
        pp_mm = ep(tc.tile_pool(name="pp_mm", bufs=2, space="PSUM"))
        pp_ytp = ep(tc.tile_pool(name="pp_ytp", bufs=3, space="PSUM"))
        pp_s = ep(tc.tile_pool(name="pp_s", bufs=2, space="PSUM"))
        pp_gd = ep(tc.tile_pool(name="pp_gd", bufs=1, space="PSUM"))
        if True:
            # ---------- constants / persistent ----------
            wt_sb = const.tile([128, 8, 2096], f8)
            nc.sync.dma_start(wt_sb, wt[:, :, :])
            mft_sb = const.tile([128, 8, DM], f8)
            nc.sync.dma_start(mft_sb, mft[:, :, :])
            cw_sb = const.tile([128, 36], f32)
            nc.sync.dma_start(cw_sb, cw[:, :])
            cb_sb = const.tile([128, 18], f32)
            nc.sync.dma_start(cb_sb, cb[:, :])
            hp_sb = const.tile([128, 18], f32)
            nc.sync.dma_start(hp_sb, hp[:, :])

            ident_b = const.tile([128, 128], bf16)
            make_identity(nc, ident_b)
            ident_f = const.tile([128, 128], f32)
            make_identity(nc, ident_f)
            # multiplicative mask, [j, i] coords: 1 where i >= j, 0 where i < j
            tril01 = const.tile([128, 128], bf16)
            nc.gpsimd.memset(tril01, 1.0)
            nc.gpsimd.affine_select(
                out=tril01, in_=tril01, compare_op=mybir.AluOpType.is_ge,
                fill=0.0, base=0, pattern=[[1, 128]], channel_multiplier=-1,
            )
            onesq = const.tile([128, 128], f32)
            nc.vector.memset(onesq, 1.0)
            onescol = const.tile([128, 1], f8)
            nc.vector.memset(onescol, 1.0)
            # conv diagonal weight tiles (fp8), built from cw columns
            convd = const.tile([128, 36, 128], f8)
            for j in range(36):
                nc.scalar.mul(convd[:, j, :], ident_b, cw_sb[:, j : j + 1])
            # persistent scan state: 4 head-groups (heads 4t+k at partitions
            # 32k..32k+16), ping-pong A/B
            stA = statep.tile([128, 4, P], f8, tag="stA")
            stB = statep.tile([128, 4, P], f8, tag="stB")
            nc.vector.memset(stA, 0.0)
            nc.vector.memset(stB, 0.0)
            st = [stA, stB]
            # chunk-decay per-partition scalars, [state-tile, chunk]
            texp_st = statep.tile([128, 4, NCH], f32, tag="texp")
            nc.vector.memset(texp_st, 0.0)

            # whole-sequence activations
            xsr = seqp.tile([128, 8, L + 3], f8)  # in_proj xs (+3 zero halo)
            bcr = seqp.tile([32, L + 3], f8)
            xs_sb = seqp.tile([128, 8, L], bf16)  # conv+silu out
            bct = seqp.tile([32, L], bf16)
            ct4 = seqp.tile([128, L], bf16)  # C rows replicated to 4 bases
            zs = seqp.tile([128, 8, L], f8)  # silu(z)
            c2t = seqp.tile([128, 4, L], f8)  # C * exp(l_h) per head
            nc.vector.memset(xsr[:, :, 0:3], 0.0)
            nc.vector.memset(bcr[:, 0:3], 0.0)

            dscr = dscrp.tile([48, L], f32, tag="dscr")
            dscr2 = dscrp.tile([16, L], bf16, tag="dscr2")
            dscr3 = dscrp.tile([32, L], f16, tag="dscr3")

            # ================= phase 1: in_proj + dt =================
            for b in range(NBLK):
                bsl = slice(b * BLK, (b + 1) * BLK)
                bsl3 = slice(3 + b * BLK, 3 + (b + 1) * BLK)
                xtb = xtp.tile([128, 8, BLK], f8, tag="xtb")
                nc.sync.dma_start(xtb, xt[:, :, bsl])
                dt_sp = dtlp.tile([128, BLK], f32, tag="dtsp")
                for et in range(9):
                    m = 128 if et < 8 else 48
                    ecol = et * 128 if et < 8 else 1024
                    ps = [pp_mm, pp_ytp][et % 2].tile(
                        [128, BLK], f32, tag=["mm", "ytp"][et % 2])
                    for kj in range(4):
                        nc.tensor.matmul(
                            ps[:m, :], wt_sb[:, 2 * kj : 2 * kj + 2, ecol : ecol + m],
                            xtb[:, 2 * kj : 2 * kj + 2, :],
                            start=(kj == 0), stop=(kj == 3), perf_mode=DR,
                        )
                    if et < 8:
                        if et % 2 == 0:
                            nc.scalar.mul(xsr[:, et, bsl3], ps, 0.0625)
                        else:
                            nc.vector.tensor_scalar_mul(
                                xsr[:, et, bsl3], ps, 0.0625)
                    else:
                        nc.scalar.mul(bcr[:, bsl3], ps[0:32, :], 0.0625)
                        nc.scalar.activation(
                            dt_sp[32:48, :], ps[32:48, :], AF.Exp,
                            bias=hp_sb[32:48, 0:1], scale=0.0625,
                        )
                        nc.vector.tensor_scalar_add(
                            dt_sp[32:48, :], dt_sp[32:48, :], 1.0
                        )
                        nc.scalar.activation(dt_sp[32:48, :], dt_sp[32:48, :], AF.Ln)
                # ---------- dt pipeline ----------
                lndt = dtlp.tile([128, BLK], f32, tag="lndt")
                lcm = dtlp.tile([128, BLK], f32, tag="lcm")
                wc2 = dtlp.tile([128, BLK], bf16, tag="wc2")
                nc.scalar.activation(lndt[32:48, :], dt_sp[32:48, :], AF.Ln)
                dtA = dtlp.tile([128, BLK], f32, tag="dtA")
                nc.vector.tensor_scalar_mul(
                    dtA[32:48, :], dt_sp[32:48, :], hp_sb[32:48, 1:2]
                )
                for cc in range(CPB):
                    qs = slice(cc * Q, (cc + 1) * Q)
                    nc.vector.tensor_tensor_scan(
                        lcm[32:48, qs], onesq[32:48, :], dtA[32:48, qs],
                        0.0, alu.mult, alu.add,
                    )
                nc.scalar.activation(wc2[32:48, :], lcm[32:48, :], AF.Exp)
                texp_cm = dtlp.tile([128, CPB, 1], f32, tag="texpcm")
                lv = lcm[32:48, :].rearrange("p (c q) -> p c q", q=Q)
                nc.scalar.activation(texp_cm[32:48, :, :], lv[:, :, 127:128], AF.Exp)
                # chunk-recentered l rows in fp16 for the gdiff matmuls
                lcmr = dtlp.tile([128, BLK], f16, tag="lcmr")
                lsubr = dtlp.tile([128, BLK], f16, tag="lsubr")
                nc.vector.tensor_tensor(
                    lcmr[32:48, :].rearrange("p (c q) -> p c q", q=Q),
                    lv, lv[:, :, 127:128].to_broadcast([16, CPB, Q]),
                    alu.subtract,
                )
                nc.vector.tensor_tensor(
                    lsubr[32:48, :], lcmr[32:48, :], lndt[32:48, :], alu.subtract
                )
                # bounce small per-block vectors through DRAM so they can be
                # partition-broadcast on the way back in
                nc.sync.dma_start(dscr3[0:16, bsl], lcmr[32:48, :])
                nc.sync.dma_start(dscr3[16:32, bsl], lsubr[32:48, :])
                nc.sync.dma_start(dscr2[:, bsl], wc2[32:48, :])
                nc.sync.dma_start(
                    dscr[32:48, b * CPB : (b + 1) * CPB],
                    texp_cm[32:48, :, :].rearrange("p c one -> p (c one)"),
                )
                for k in range(4):
                    nc.sync.dma_start(
                        texp_st[32 * k : 32 * k + 16, :, b * CPB : (b + 1) * CPB],
                        bass.AP(dscr.tensor,
                                dscr.offset + (32 + k) * L + b * CPB,
                                [[0, 16], [4 * L, 4], [1, CPB]]),
                    )

            # ================= phase 2: z + conv (all silus) =================
            for b in range(NBLK):
                bsl = slice(b * BLK, (b + 1) * BLK)
                xtb = xtp.tile([128, 8, BLK], f8, tag="xtb")
                nc.sync.dma_start(xtb, xt[:, :, bsl])
                for zt in range(8):
                    ps = [pp_mm, pp_ytp][zt % 2].tile(
                        [128, BLK], f32, tag=["mm", "ytp"][zt % 2])
                    for kj in range(4):
                        nc.tensor.matmul(
                            ps, wt_sb[:, 2 * kj : 2 * kj + 2, 1072 + zt * 128 : 1200 + zt * 128],
                            xtb[:, 2 * kj : 2 * kj + 2, :],
                            start=(kj == 0), stop=(kj == 3), perf_mode=DR,
                        )
                    nc.scalar.activation(zs[:, zt, bsl], ps, AF.Silu, scale=0.0625)
                for ct in range(9):
                    m = 128 if ct < 8 else 32
                    ps = [pp_mm, pp_ytp][ct % 2].tile(
                        [128, BLK], f32, tag=["mm", "ytp"][ct % 2])
                    for k in range(4):
                        a = b * BLK + k
                        mov = (xsr[:, ct, a : a + BLK] if ct < 8
                               else bcr[:, a : a + BLK])
                        nc.tensor.matmul(
                            ps[:m, :], convd[:m, ct * 4 + k, :m], mov,
                            start=(k == 0), stop=(k == 3),
                        )
                    dst = xs_sb[:, ct, bsl] if ct < 8 else bct[:, bsl]
                    nc.scalar.activation(
                        dst, ps[:m, :], AF.Silu, bias=cb_sb[:m, ct : ct + 1]
                    )
                # C rows replicated to the four 32-aligned bases
                for k4 in range(4):
                    nc.sync.dma_start(ct4[32 * k4 : 32 * k4 + 16, bsl], bct[16:32, bsl])
                # C'' = C * exp(l_h) per head
                wc2rep = wcrp.tile([128, 4, BLK], bf16, tag="wc2rep")
                _qw = [nc.scalar, nc.gpsimd, nc.sync, nc.gpsimd]
                for k in range(4):
                    _qw[k].dma_start(
                        wc2rep[32 * k : 32 * k + 16, :, :],
                        bass.AP(dscr2.tensor, dscr2.offset + k * L + b * BLK,
                                [[0, 16], [4 * L, 4], [1, BLK]]),
                    )
                nc.vector.tensor_tensor(
                    c2t[:, :, bsl],
                    ct4[:, bsl].rearrange("p (one c) -> p one c", one=1)
                    .to_broadcast([128, 4, BLK]),
                    wc2rep, alu.mult,
                )

            # ================= phase 3: scan chunks =================
            for c in range(NCH):
                cc = c % CPB
                b = c // CPB
                qs = slice(c * Q, (c + 1) * Q)
                if cc == 0:
                    y2 = y2blk.tile([128, 8, BLK], f8, tag="y2")
                # xpos: PE-transpose xs chunk to position-major
                xposr = chkp.tile([128, 8, Q], bf16, tag="xposr")
                for w in range(2):
                    tp = pp_ytp.tile([128, 512], f32, tag="ytp")
                    tpb = tp.bitcast(bf16)
                    for ct in range(4):
                        nc.tensor.transpose(
                            tpb[:, ct * 128 : ct * 128 + 128],
                            xs_sb[:, w * 4 + ct, qs], ident_b,
                        )
                    nc.scalar.copy(xposr[:, w * 4 : w * 4 + 4, :], tpb[:, 0:512])
                xpv = xposr.rearrange("p t c -> p (t c)").rearrange(
                    "p (h c) -> p h c", c=P
                )
                # S psum + B transpose share one bank
                sps = pp_s.tile([128, 512], f32, tag="sps")
                nc.vector.memset(sps[:, 0:256], 0.0)
                # B position-major (bf16 view of spare sps columns)
                bpp = sps.bitcast(bf16)
                nc.tensor.transpose(bpp[:, 576:592], bct[0:16, qs], ident_b[0:16, 0:16])
                bpos = chkp.tile([128, NST], bf16, tag="bpos")
                nc.vector.tensor_copy(bpos, bpp[:, 576:592])
                # head-shared C@B^T -> G0^T[j, i], masked below the diagonal
                nc.tensor.matmul(
                    sps[:, 384:512], bct[0:16, qs], ct4[0:16, qs],
                    start=True, stop=True,
                )
                g0m = chkp.tile([128, Q], bf16, tag="g0m")
                nc.vector.tensor_tensor(g0m, sps[:, 384:512], tril01, alu.mult)
                # gdiff[j,i] = l_i - l_j + ln dt_j via K=2 fp16 matmuls
                # (chunk-recentered rows from dscr3; the shift cancels).
                # exp overflows to +inf above the diagonal; min(.,100) then
                # the g0m mask zeroes those entries.
                lsg = lsgp.tile([2, NH, Q], f16, tag="lsg")  # p0: data, p1: 1
                rsg = lsgp.tile([2, NH, Q], f16, tag="rsg")  # p0: -1, p1: data
                if c < 2:
                    nc.gpsimd.memset(lsg, 1.0)
                    nc.gpsimd.memset(rsg, -1.0)
                nc.gpsimd.dma_start(
                    lsg.rearrange("p h q -> p (h q)")[0:1, :], dscr3[16:32, qs]
                )
                nc.gpsimd.dma_start(
                    rsg.rearrange("p h q -> p (h q)")[1:2, :], dscr3[0:16, qs]
                )
                gdec = gdecp.tile([128, NH, Q], bf16, tag="gdec")
                gsb = gsbp.tile([128, NH, Q], bf16, tag="gsb")
                for g in range(4):
                    gdp = pp_gd.tile([128, 512], f32, tag="gd")
                    for hh_ in range(4):
                        h = g * 4 + hh_
                        lhsT = bass.AP(
                            lsg.tensor, lsg.offset + h * Q,
                            [[NH * Q, 2], [1, Q]],
                        )
                        rhs = bass.AP(
                            rsg.tensor, rsg.offset + h * Q,
                            [[NH * Q, 2], [1, Q]],
                        )
                        nc.tensor.matmul(
                            gdp[:, hh_ * 128 : hh_ * 128 + 128], lhsT, rhs,
                            start=True, stop=True, tile_position=(0, 0),
                        )
                    hsl = slice(g * 4, g * 4 + 4)
                    nc.scalar.activation(gdec[:, hsl, :], gdp, AF.Exp)
                    nc.vector.scalar_tensor_tensor(
                        gsb[:, hsl, :], gdec[:, hsl, :], 100.0,
                        g0m.rearrange("p (one i) -> p one i", one=1)
                        .to_broadcast([128, 4, Q]),
                        alu.min, alu.mult,
                    )
                # wS column = gdec[:, :, last] = dt_j exp(T - l_j); B_ws
                bws = chkp.tile([128, NH, NST], bf16, tag="bws")
                nc.vector.tensor_tensor(
                    bws,
                    bpos.rearrange("p (one n) -> p one n", one=1).to_broadcast([128, NH, NST]),
                    gdec[:, :, 127:128].to_broadcast([128, NH, NST]),
                    alu.mult,
                )
                # Y psums: pairs 0-3 and 4-7
                yA = pp_ytp.tile([128, 512], f32, tag="ytp")
                yB = pp_ytp.tile([128, 512], f32, tag="ytp")
                ypair = [yA, yB]
                for h in range(NH):
                    k, t = h % 4, h // 4
                    # S^T = B_ws.T @ X_h
                    nc.tensor.matmul(
                        sps[32 * k : 32 * k + 16, t * 64 : t * 64 + 64],
                        bws[:, h, :], xpv[:, h, :],
                        start=True, stop=True, tile_position=(0, 32 * k),
                    )
                for h in range(NH):
                    k, t = h % 4, h // 4
                    pr = h // 2
                    # Y^T = X_h.T @ G^T (+ H^T.T @ C''^T)
                    yp = ypair[pr // 4]
                    ysl = (slice(64 * (h % 2), 64 * (h % 2) + 64),
                           slice((pr % 4) * 128, (pr % 4) * 128 + 128))
                    nc.tensor.matmul(
                        yp[ysl[0], ysl[1]], xpv[:, h, :], gsb[:, h, :],
                        start=True, stop=(c == 0),
                    )
                    if c > 0:
                        nc.tensor.matmul(
                            yp[ysl[0], ysl[1]],
                            st[c % 2][32 * k : 32 * k + 16, t, :],
                            c2t[32 * k : 32 * k + 16, t, qs],
                            start=False, stop=True,
                            tile_position=(32 * k, 64 * (h % 2)),
                        )
                # state recurrence
                for t in range(4):
                    nc.vector.scalar_tensor_tensor(
                        st[(c + 1) % 2][:, t, :], st[c % 2][:, t, :],
                        texp_st[:, t, c : c + 1], sps[:, t * 64 : t * 64 + 64],
                        alu.mult, alu.add,
                    )
                # ---------- gating (y8 = gated-y/8 in fp8) ----------
                y8 = y8p.tile([128, 8, Q], f8, tag="y8")
                tmp = gp2.tile([128, 4, Q], f32, tag="gtmp")
                for half in range(2):
                    yp = ypair[half]
                    hsl4 = slice(half * 4, half * 4 + 4)
                    for pr4 in range(4):
                        pr = half * 4 + pr4
                        nc.vector.scalar_tensor_tensor(
                            tmp[:, pr4, :], xs_sb[:, pr, qs],
                            hp_sb[:, 2 + pr : 3 + pr],
                            yp[:, pr4 * 128 : pr4 * 128 + 128],
                            alu.mult, alu.add,
                        )
                    nc.vector.scalar_tensor_tensor(
                        y8[:, hsl4, :], tmp, 0.125, zs[:, hsl4, qs],
                        alu.mult, alu.mult,
                    )
                    nc.gpsimd.tensor_tensor(
                        y2[:, hsl4, cc * Q : cc * Q + Q], y8[:, hsl4, :],
                        y8[:, hsl4, :], alu.mult,
                    )
                # ---------- U matmuls (fp8 DoubleRow) + store ----------
                for oc in range(2):
                    ups = pp_mm.tile([128, BLK], f32, tag="mm")
                    for cj in range(4):
                        nc.tensor.matmul(
                            ups, y8[:, 2 * cj : 2 * cj + 2, :],
                            mft_sb[:, 2 * cj : 2 * cj + 2, oc * 512 : oc * 512 + 512],
                            start=(cj == 0), stop=(cj == 3), perf_mode=DR,
                        )
                    usb = y2p.tile([128, BLK], bf16, tag="usb")
                    nc.scalar.copy(usb, ups)
                    nc.gpsimd.dma_start(
                        u[c * 128 : c * 128 + 128, oc * 512 : oc * 512 + 512], usb
                    )
                # block sumsq
                if cc == CPB - 1:
                    ssps = pp_mm.tile([128, BLK], f32, tag="mm")
                    for ct in range(8):
                        nc.tensor.matmul(
                            ssps[0:1, :], onescol, y2[:, ct, :],
                            start=(ct == 0), stop=(ct == 7),
                        )
                    ssb = y2p.tile([1, BLK], f32, tag="ssb")
                    nc.vector.tensor_copy(ssb, ssps[0:1, :])
                    nc.sync.dma_start(s[0:1, b * BLK : (b + 1) * BLK], ssb)

    nc.finalize()
    return nc


def _get_nc():
    if "nc" not in _cache:
        _cache["nc"] = _build_nc()
    return _cache["nc"]


def _prep_core_inputs(inputs, b, d, hh):
    import ml_dtypes

    fp8 = ml_dtypes.float8_e4m3

    def to8(a):
        return np.clip(a, -240.0, 240.0).astype(fp8)

    pre = "fwd" if d == 0 else "bwd"
    W = np.asarray(inputs[f"{pre}_in_proj_w"], np.float32)  # (4160, 1024)
    x = np.asarray(inputs["x"], np.float32)[b]  # (L, 1024)
    if d == 1:
        x = x[::-1]
    # x^T as (128, 8, L)
    xtv = np.ascontiguousarray(x.T.reshape(8, 128, L).transpose(1, 0, 2))
    # in_proj^T columns: [xs 1024 | B 16 | C 16 | dt 16 | z 1024], x16 for fp8
    W_xs = W[2048 + hh * 1024 : 3072 + hh * 1024]
    W_B = W[4096:4112]
    W_C = W[4112:4128]
    W_dt = W[4128 + hh * 16 : 4144 + hh * 16]
    W_z = W[hh * 1024 : 1024 + hh * 1024]
    Wt = np.concatenate([W_xs, W_B, W_C, W_dt, W_z], axis=0).T * 16.0  # (1024, 2096)
    wtv = np.ascontiguousarray(Wt.reshape(8, 128, 2096).transpose(1, 0, 2))
    # fused output projection, x64 for fp8; y8 carries 1/8 -> U is 8x
    Wo = np.asarray(inputs[f"{pre}_out_proj_w"], np.float32)  # (1024, 2048)
    Wl = np.asarray(inputs["layer_out_proj_w"], np.float32)  # (1024, 2048)
    nw = np.asarray(inputs[f"{pre}_norm_w"], np.float32)
    ch = slice(hh * 1024, hh * 1024 + 1024)
    M = (Wl[:, d * 1024 : d * 1024 + 1024] @ Wo)[:, ch] * nw[ch][None, :]
    MfT = M.T * 64.0  # (1024 c, 1024 o)
    mftv = np.ascontiguousarray(MfT.reshape(8, 128, 1024).transpose(1, 0, 2))
    # conv: diag values per (c-tile, tap) column
    cwf = np.asarray(inputs[f"{pre}_conv_w"], np.float32)[:, 0, :]  # (2080, 4)
    cwl = np.concatenate([cwf[hh * 1024 : 1024 + hh * 1024], cwf[2048:2080]], axis=0)
    cwv = np.zeros((128, 36), np.float32)
    for ct in range(9):
        n = 128 if ct < 8 else 32
        for k in range(4):
            cwv[:n, ct * 4 + k] = cwl[ct * 128 : ct * 128 + n, k]
    cbf = np.asarray(inputs[f"{pre}_conv_b"], np.float32)
    cbl = np.concatenate([cbf[hh * 1024 : 1024 + hh * 1024], cbf[2048:2080]])
    cbv = np.zeros((128, 18), np.float32)
    for ct in range(9):
        n = 128 if ct < 8 else 32
        cbv[:n, ct] = cbl[ct * 128 : ct * 128 + n]
    # host params
    hpv = np.zeros((128, 18), np.float32)
    hs = slice(hh * 16, hh * 16 + 16)
    hpv[32:48, 0] = np.asarray(inputs[f"{pre}_dt_bias"], np.float32)[hs]
    hpv[32:48, 1] = -np.exp(np.asarray(inputs[f"{pre}_A_log"], np.float32)[hs])
    Dp = np.asarray(inputs[f"{pre}_Dp"], np.float32)[hs]
    for h in range(NH):
        hpv[:, 2 + h] = Dp[h]
    return {
        "xt": to8(xtv),
        "wt": to8(wtv),
        "mft": to8(mftv),
        "cw": cwv,
        "cb": cbv,
        "hp": hpv,
    }


def _combine(inputs, results):
    x = np.asarray(inputs["x"], np.float32)
    scale = np.asarray(inputs["layer_scale"], np.float32)
    out = x.copy()
    i = 0
    for b in range(2):
        for d in range(2):
            U0 = np.asarray(results[i]["u"], np.float32)
            s0 = results[i]["s"][0]
            U1 = np.asarray(results[i + 1]["u"], np.float32)
            s1 = results[i + 1]["s"][0]
            i += 2
            r = 1.0 / np.sqrt((s0 + s1) / 32.0 + EPS)
            contrib = r[:, None] * (U0 + U1) / 8.0
            if d == 1:
                contrib = contrib[::-1]
            out[b] += contrib * scale[None, :]
    return out


def _run(inputs, trace=False, core_ids=None):
    from concourse.bass_utils import run_bass_kernel_spmd

    nc = _get_nc()
    in_maps = []
    for b in range(2):
        for d in range(2):
            for hh in range(2):
                in_maps.append(_prep_core_inputs(inputs, b, d, hh))
    if core_ids is None:
        core_ids = list(range(8))
    res = run_bass_kernel_spmd(
        nc, in_maps[: len(core_ids)], core_ids=core_ids, trace=trace
    )
    return res


def kernel(**inputs):
    res = _run(inputs)
    return _combine(inputs, res.results)
